# revision 28
# baseline (speedup 1.0000x reference)
"""Trainium2 Bass kernel for nn_AffineTransformer_6442450944616.

kernel(**inputs): FULL inputs -> (fill_out, stroke_out) [2048,128,128] f32,
matching reference.reference().  Data-parallel over samples, 8 cores.

Wall time under axon is dominated by host<->device transfer (~55-80 MB/s,
half-duplex tunnel), so the kernel minimizes transferred bytes and
overlaps everything it can:
  - images are sent as PACKED 6-bit codes (x63, 4 px per 3 bytes),
    12.6MB instead of 67MB f32; the device unpacks with strided bitwise
    DVE ops and the 255/63 rescale is folded into the output convert
    (rel err 6.1e-3 vs tolerance 2e-2, validated by simulation)
  - the output is SPARSE-COMPACTED on device: only quarter-rows (32px)
    whose bilinear-warp support is nonempty are downloaded.  The support
    is an interval per output row, computed exactly on the host from the
    affine params; the host uploads just the keep FLAGS ([ns,128,4]
    u8, map-independent) plus a per-sample slot base, and the device
    builds the slot indices itself (triangular-matmul prefix sum across
    partitions + Hillis-Steele scan over the 8 groups), then scatters
    kept quarter-rows into a compact [XSLOTS,24] uint8 tensor with
    indirect DMAs (out-of-range index = dropped slot).
    ~25% of slots are kept -> ~19MB download instead of 268MB f32 dense.
    If a core's kept slots exceed the static capacity, the overflow
    slots are computed on the host (exact, vectorized) — a rarely-taken
    safety net for input-distribution shift.
  - output values are 6-bit codes (x63), packed 4 px per 3 bytes on
    device -> download is 14.2MB; total rel err ~1.05e-2 vs tol 2e-2
  - the batch runs as two half-batch device calls (ns=128/core each) so
    device exec (dominated by ~1.2us/descriptor indirect-DMA processing)
    of one half hides under the tunnel stream of the other
  - affine params are sent as [ns,8] f32 and expanded on device; pj/qj
    pixel grids are generated on-device with iota; output donation
    buffers are created on-device (run_bass_kernel_spmd would upload
    them as host zeros every call) via a runner modeled on
    bass2jax.run_bass_via_pjrt with a cached jitted callable
  - image upload is dispatched per core-shard as soon as each shard is
    quantized; fetch+dequant+reconstruct run per shard so host work
    overlaps the tunnel stream

Math per sample i, pixel j (p=j//128, q=j%128):
  ix(j)=t00*q+t01*p+Cx ; iy likewise
  out[j] = sum_{x,y payload} relu(1-|ix-x|) * relu(1-|iy-y|) * img[y,x]
(exact bilinear-with-zeros; hat weights equal (1-w, w) on live taps).
A pixel can be nonzero only if ix in (-1,64) and iy in (-1,64); for
fixed p both are linear in q, so the support is a q-interval per row ->
the host knows exactly which quarter-rows matter (eps-margined for f32).
"""
import numpy as np
import jax
import jax.numpy as jnp
from jax.sharding import Mesh, NamedSharding, PartitionSpec
from jax.experimental.shard_map import shard_map

import concourse.bass as bass
import concourse.bacc as bacc
import concourse.tile as tile
import concourse.mybir as mybir
from concourse import bass2jax

F32 = mybir.dt.float32
BF16 = mybir.dt.bfloat16
I32 = mybir.dt.int32
U8 = mybir.dt.uint8
U16 = mybir.dt.uint16
AL = mybir.AluOpType
ACTF = mybir.ActivationFunctionType

N = 2048
NCORES = 8
NHALF = 2               # split the batch into 2 pipelined device calls
NS = N // NCORES // NHALF   # 128 samples per core per call
P = 128
NPIX = P * P
CH = 1024
NCH = NPIX // CH
XSLOTS = 36864          # compact quarter-row slot capacity per core-call
EPS = 0.05              # support-interval widening (covers f32 rounding)


def _build(ns: int):
    nc = bacc.Bacc("TRN2", target_bir_lowering=False, debug=False)
    ibt_d = nc.dram_tensor("ibt", [ns, 64, 96], U8, kind="ExternalInput")
    wc6_d = nc.dram_tensor("wc6", [ns, 8], F32, kind="ExternalInput")
    flg_d = nc.dram_tensor("flags", [ns, P, 4], U8, kind="ExternalInput")
    comp_d = nc.dram_tensor("comp", [XSLOTS, 24], U8, kind="ExternalOutput")

    with tile.TileContext(nc) as tc:
        with tc.tile_pool(name="const", bufs=1) as cpool, \
             tc.tile_pool(name="work", bufs=3) as pool, \
             tc.tile_pool(name="out", bufs=2) as opool, \
             tc.tile_pool(name="ps", bufs=2, space="PSUM") as psum, \
             tc.tile_pool(name="psw", bufs=1, space="PSUM") as psumw:
            # on-device constants: local pixel grids, chunk offsets,
            # per-partition p%64, matmul helper matrices
            pj0i = cpool.tile([P, CH], I32, tag="pj0i")
            qj0i = cpool.tile([P, CH], I32, tag="qj0i")
            c8i = cpool.tile([P, NCH], I32, tag="c8i")
            pm64i = cpool.tile([P, 1], I32, tag="pm64i")
            nc.gpsimd.iota(pj0i[:], pattern=[[1, 8], [0, P]], base=0,
                           channel_multiplier=0)
            nc.gpsimd.iota(qj0i[:], pattern=[[0, 8], [1, P]], base=0,
                           channel_multiplier=0)
            nc.gpsimd.iota(c8i[:], pattern=[[8, NCH]], base=0,
                           channel_multiplier=0)
            nc.gpsimd.iota(pm64i[0:64, :], pattern=[[0, 1]], base=0,
                           channel_multiplier=1)
            nc.gpsimd.iota(pm64i[64:128, :], pattern=[[0, 1]], base=0,
                           channel_multiplier=1)
            pj0 = cpool.tile([P, CH], F32, tag="pj0")
            qj0 = cpool.tile([P, CH], F32, tag="qj0")
            c8 = cpool.tile([P, NCH], F32, tag="c8")
            pm64 = cpool.tile([P, 1], F32, tag="pm64")
            nc.scalar.copy(out=pj0[:], in_=pj0i[:])
            nc.scalar.copy(out=qj0[:], in_=qj0i[:])
            nc.scalar.copy(out=c8[:], in_=c8i[:])
            nc.scalar.copy(out=pm64[:], in_=pm64i[:])
            ones2 = cpool.tile([P, 2], BF16, tag="ones2")
            nc.vector.memset(ones2[:], 0.0)
            nc.vector.memset(ones2[0:64, 0:1], 1.0)
            nc.vector.memset(ones2[64:128, 1:2], 1.0)
            one1 = cpool.tile([1, P], F32, tag="one1")
            nc.vector.memset(one1[:], 1.0)
            # triS[k,p] = (p > k): strict lower prefix matmul operand
            pfree_i = cpool.tile([P, P], I32, tag="pfree_i")
            kv_i = cpool.tile([P, 1], I32, tag="kv_i")
            nc.gpsimd.iota(pfree_i[:], pattern=[[1, P]], base=0,
                           channel_multiplier=0)
            nc.gpsimd.iota(kv_i[:], pattern=[[0, 1]], base=0,
                           channel_multiplier=1)
            pfree = cpool.tile([P, P], F32, tag="pfree")
            kv = cpool.tile([P, 1], F32, tag="kv")
            nc.scalar.copy(out=pfree[:], in_=pfree_i[:])
            nc.scalar.copy(out=kv[:], in_=kv_i[:])
            triS = cpool.tile([P, P], F32, tag="triS")
            nc.vector.tensor_scalar(triS[:], pfree[:], kv[:, 0:1], None,
                                    AL.is_gt)

            with tc.For_i(0, ns, 1) as i:
                w6 = pool.tile([1, 8], F32, tag="w6", name=f"w6{i}")
                ibtu = pool.tile([64, 96], U8, tag="ibtu", name=f"ibtu{i}")
                ftile = pool.tile([P, 4], U8, tag="ftile", name=f"ft{i}")
                nc.sync.dma_start(out=w6[:], in_=wc6_d[bass.ds(i, 1), :])
                nc.sync.dma_start(out=ibtu[:], in_=ibt_d[bass.ds(i, 1), :, :])
                nc.sync.dma_start(out=ftile[:], in_=flg_d[bass.ds(i, 1), :, :])
                # unpack 6-bit codes: 3 bytes -> 4 codes, strided views
                codes = pool.tile([64, P], U8, tag="codes", name=f"cd{i}")
                b0 = ibtu[:, 0:96:3]; b1 = ibtu[:, 1:96:3]; b2 = ibtu[:, 2:96:3]
                nc.vector.tensor_scalar(codes[:, 0:128:4], b0, 63, None,
                                        AL.bitwise_and)
                t1 = pool.tile([64, 32], U8, tag="t1", name=f"t1{i}")
                nc.vector.tensor_scalar(t1[:], b0, 6, None,
                                        AL.logical_shift_right)
                t2 = pool.tile([64, 32], U8, tag="t2", name=f"t2{i}")
                nc.vector.tensor_scalar(t2[:], b1, 15, 2, AL.bitwise_and,
                                        AL.logical_shift_left)
                nc.vector.tensor_tensor(codes[:, 1:128:4], t1[:], t2[:],
                                        AL.bitwise_or)
                t3 = pool.tile([64, 32], U8, tag="t3", name=f"t3{i}")
                nc.vector.tensor_scalar(t3[:], b1, 4, None,
                                        AL.logical_shift_right)
                t4 = pool.tile([64, 32], U8, tag="t4", name=f"t4{i}")
                nc.vector.tensor_scalar(t4[:], b2, 3, 4, AL.bitwise_and,
                                        AL.logical_shift_left)
                nc.vector.tensor_tensor(codes[:, 2:128:4], t3[:], t4[:],
                                        AL.bitwise_or)
                nc.vector.tensor_scalar(codes[:, 3:128:4], b2, 2, None,
                                        AL.logical_shift_right)
                ibtf = pool.tile([64, P], BF16, tag="ibtf", name=f"ibtf{i}")
                nc.scalar.copy(out=ibtf[:], in_=codes[:])
                # broadcast w6 row to all partitions, then select per-half
                wcb = psumw.tile([P, 8], F32, tag="wcb", name=f"wcb{i}")
                nc.tensor.matmul(out=wcb[:], lhsT=one1[:], rhs=w6[:],
                                 start=True, stop=True)
                wcs = pool.tile([P, 8], F32, tag="wcs", name=f"wcs{i}")
                nc.scalar.copy(out=wcs[:], in_=wcb[:])
                wcf0 = pool.tile([P, 1], F32, tag="wcf0", name=f"wcf0{i}")
                nc.scalar.copy(out=wcf0[0:64, :], in_=wcs[0:64, 0:1])
                nc.scalar.copy(out=wcf0[64:128, :], in_=wcs[64:128, 3:4])
                wcf1 = pool.tile([P, 1], F32, tag="wcf1", name=f"wcf1{i}")
                nc.scalar.copy(out=wcf1[0:64, :], in_=wcs[0:64, 1:2])
                nc.scalar.copy(out=wcf1[64:128, :], in_=wcs[64:128, 4:5])
                wc2f = pool.tile([P, 1], F32, tag="wc2f", name=f"wc2f{i}")
                nc.scalar.activation(out=wc2f[0:64, :], in_=pm64[0:64, :],
                                     func=ACTF.Identity, scale=-1.0,
                                     bias=wcs[0:64, 2:3])
                nc.scalar.activation(out=wc2f[64:128, :], in_=pm64[64:128, :],
                                     func=ACTF.Identity, scale=-1.0,
                                     bias=wcs[64:128, 5:6])
                # build slot indices from flags: idx(p,g) enumerates
                # (p-major, g=4m+qh) within the sample, plus the
                # per-sample base carried in wc6 col 6
                btile = pool.tile([P, 8], F32, tag="btile", name=f"bt{i}")
                nc.scalar.copy(out=btile[:, 0:4], in_=ftile[:])
                nc.scalar.copy(out=btile[:, 4:8], in_=ftile[:])
                scr8 = pool.tile([P, 8], F32, tag="scr8", name=f"sc{i}")
                rowsum = pool.tile([P, 1], F32, tag="rowsum", name=f"rs{i}")
                nc.vector.tensor_scalar(scr8[:], btile[:], 1.0, 0.0, AL.mult,
                                        AL.add, accum_out=rowsum[:])
                aux = psumw.tile([P, 1], F32, tag="aux", name=f"aux{i}")
                nc.tensor.matmul(out=aux[:], lhsT=triS[:], rhs=rowsum[:],
                                 start=True, stop=True)
                rowbase = pool.tile([P, 1], F32, tag="rowbase", name=f"rb{i}")
                nc.scalar.copy(out=rowbase[:], in_=aux[:])
                s1 = pool.tile([P, 8], F32, tag="s1", name=f"s1{i}")
                nc.vector.tensor_tensor(s1[:, 1:8], btile[:, 1:8],
                                        btile[:, 0:7], AL.add)
                nc.scalar.copy(out=s1[:, 0:1], in_=btile[:, 0:1])
                s2 = pool.tile([P, 8], F32, tag="s2", name=f"s2{i}")
                nc.vector.tensor_tensor(s2[:, 2:8], s1[:, 2:8], s1[:, 0:6],
                                        AL.add)
                nc.scalar.copy(out=s2[:, 0:2], in_=s1[:, 0:2])
                s4 = pool.tile([P, 8], F32, tag="s4", name=f"s4{i}")
                nc.vector.tensor_tensor(s4[:, 4:8], s2[:, 4:8], s2[:, 0:4],
                                        AL.add)
                nc.scalar.copy(out=s4[:, 0:4], in_=s2[:, 0:4])
                excl = pool.tile([P, 8], F32, tag="excl", name=f"ex{i}")
                nc.vector.tensor_tensor(excl[:], s4[:], btile[:], AL.subtract)
                idxf = pool.tile([P, 8], F32, tag="idxf", name=f"ix{i}")
                nc.vector.tensor_scalar(idxf[:], excl[:], rowbase[:, 0:1],
                                        wcs[:, 6:7], AL.add, AL.add)
                msk = pool.tile([P, 8], F32, tag="msk", name=f"mk{i}")
                nc.vector.scalar_tensor_tensor(msk[:], idxf[:], float(XSLOTS),
                                               btile[:], AL.subtract, AL.mult)
                sfin = pool.tile([P, 8], F32, tag="sfin", name=f"sf{i}")
                nc.vector.tensor_scalar(sfin[:], msk[:], float(XSLOTS), None,
                                        AL.add)
                stile32 = pool.tile([P, 8], I32, tag="stile32", name=f"s32{i}")
                nc.scalar.copy(out=stile32[:], in_=sfin[:])
                # per-chunk abs-bias: wc2f + 8c*wcf0
                wc2c = pool.tile([P, NCH], F32, tag="wc2c", name=f"wc2c{i}")
                nc.scalar.activation(out=wc2c[:], in_=c8[:], func=ACTF.Identity,
                                     scale=wcf0[:], bias=wc2f[:])
                o2 = opool.tile([2, NPIX], U8, tag="o2", name=f"o2_{i}")
                for c in range(NCH):
                    d1 = pool.tile([P, CH], F32, tag="d1", name=f"d1_{c}")
                    nc.gpsimd.tensor_scalar(d1[:], pj0[:], wcf0[:], None,
                                            AL.mult)
                    d2 = pool.tile([P, CH], F32, tag="d2", name=f"d2_{c}")
                    nc.vector.scalar_tensor_tensor(d2[:], qj0[:], wcf1[:], d1[:],
                                                   AL.mult, AL.add)
                    ab = pool.tile([P, CH], F32, tag="ab", name=f"ab_{c}")
                    nc.scalar.activation(out=ab[:], in_=d2[:], func=ACTF.Abs,
                                         scale=1.0, bias=wc2c[:, c:c + 1])
                    hh = pool.tile([P, CH], BF16, tag="hh", name=f"hh_{c}")
                    nc.scalar.activation(out=hh[:], in_=ab[:], func=ACTF.Relu,
                                         scale=-1.0, bias=1.0)
                    cc = psum.tile([P, CH], F32, tag="C", name=f"cc_{c}")
                    for h in range(CH // 512):
                        hs = slice(h * 512, (h + 1) * 512)
                        nc.tensor.matmul(out=cc[:, hs], lhsT=ibtf[:], rhs=hh[0:64, hs],
                                         start=True, stop=True)
                    mm = pool.tile([P, CH], BF16, tag="mm", name=f"mm_{c}")
                    nc.vector.tensor_tensor(mm[0:64, :], cc[0:64, :], hh[64:128, :],
                                            AL.mult)
                    nc.vector.tensor_tensor(mm[64:128, :], cc[64:128, :],
                                            hh[64:128, :], AL.mult)
                    for h in range(CH // 512):
                        hs = slice(h * 512, (h + 1) * 512)
                        oo = psum.tile([2, 512], F32, tag="O", name=f"oo_{c}_{h}")
                        nc.tensor.matmul(out=oo[:], lhsT=ones2[:], rhs=mm[:, hs],
                                         start=True, stop=True)
                        # f32 PSUM -> uint8 SBUF (round-to-nearest, saturating)
                        nc.scalar.activation(out=o2[:, c * CH + h * 512:
                                                    c * CH + (h + 1) * 512],
                                             in_=oo[:], func=ACTF.Copy,
                                             scale=1.0)
                # pack 6-bit output codes: 4 px -> 3 bytes
                o2p = opool.tile([2, 12288], U8, tag="o2p", name=f"o2p_{i}")
                pc0 = o2[:, 0:NPIX:4]; pc1 = o2[:, 1:NPIX:4]
                pc2 = o2[:, 2:NPIX:4]; pc3 = o2[:, 3:NPIX:4]
                u1 = opool.tile([2, 4096], U8, tag="u1", name=f"u1_{i}")
                nc.vector.tensor_scalar(u1[:], pc1, 3, 6, AL.bitwise_and,
                                        AL.logical_shift_left)
                nc.vector.tensor_tensor(o2p[:, 0:12288:3], pc0, u1[:],
                                        AL.bitwise_or)
                u2 = opool.tile([2, 4096], U8, tag="u2", name=f"u2_{i}")
                nc.vector.tensor_scalar(u2[:], pc1, 2, None,
                                        AL.logical_shift_right)
                u3 = opool.tile([2, 4096], U8, tag="u3", name=f"u3_{i}")
                nc.vector.tensor_scalar(u3[:], pc2, 15, 4, AL.bitwise_and,
                                        AL.logical_shift_left)
                nc.vector.tensor_tensor(o2p[:, 1:12288:3], u2[:], u3[:],
                                        AL.bitwise_or)
                u4 = opool.tile([2, 4096], U8, tag="u4", name=f"u4_{i}")
                nc.vector.tensor_scalar(u4[:], pc2, 4, None,
                                        AL.logical_shift_right)
                u5 = opool.tile([2, 4096], U8, tag="u5", name=f"u5_{i}")
                nc.vector.tensor_scalar(u5[:], pc3, 2, None,
                                        AL.logical_shift_left)
                nc.vector.tensor_tensor(o2p[:, 2:12288:3], u4[:], u5[:],
                                        AL.bitwise_or)
                # reshape packed rows onto partitions:
                # o2r[p, 96*m + 24*qh + b] = map m, row p, quarter qh
                o2r = opool.tile([P, 192], U8, tag="o2r", name=f"o2r_{i}")
                nc.sync.dma_start(out=o2r[:, 0:96], in_=o2p[0:1, :])
                nc.sync.dma_start(out=o2r[:, 96:192], in_=o2p[1:2, :])
                # scatter kept quarter-rows to their compact slots; index
                # XSLOTS (> bounds) drops the slot.  g = 4*m + qh.
                for g in range(8):
                    nc.gpsimd.indirect_dma_start(
                        out=comp_d[:, :],
                        out_offset=bass.IndirectOffsetOnAxis(
                            ap=stile32[:, g:g + 1], axis=0),
                        in_=o2r[:, 24 * g:24 * (g + 1)],
                        in_offset=None,
                        bounds_check=XSLOTS - 1,
                        oob_is_err=False)
    nc.compile()
    return nc


class _Runtime:
    pass


_CACHE = {}


def _get_runtime() -> _Runtime:
    if "rt" in _CACHE:
        return _CACHE["rt"]
    nc = _build(NS)
    bass2jax.install_neuronx_cc_hook()
    assert nc.dbg_addr is None

    in_names, out_names, out_avals = [], [], []
    partition_name = (nc.partition_id_tensor.name
                      if nc.partition_id_tensor else None)
    for alloc in nc.m.functions[0].allocations:
        if not isinstance(alloc, mybir.MemoryLocationSet):
            continue
        name = alloc.memorylocations[0].name
        if alloc.kind == "ExternalInput":
            if name != partition_name:
                in_names.append(name)
        elif alloc.kind == "ExternalOutput":
            out_names.append(name)
            out_avals.append(jax.core.ShapedArray(
                tuple(alloc.tensor_shape), mybir.dt.np(alloc.dtype)))
    n_params = len(in_names)
    n_outs = len(out_names)
    full_in_names = list(in_names) + list(out_names)
    if partition_name is not None:
        full_in_names.append(partition_name)

    def _body(*args):
        operands = list(args)
        if partition_name is not None:
            operands.append(bass2jax.partition_id_tensor())
        outs = bass2jax._bass_exec_p.bind(
            *operands,
            out_avals=tuple(out_avals),
            in_names=tuple(full_in_names),
            out_names=tuple(out_names),
            lowering_input_output_aliases=(),
            sim_require_finite=True,
            sim_require_nnan=True,
            nc=nc,
        )
        return tuple(outs)

    mesh = Mesh(np.asarray(jax.devices()[:NCORES]), ("core",))
    donate = tuple(range(n_params, n_params + n_outs))
    sharded = jax.jit(
        shard_map(_body, mesh=mesh,
                  in_specs=(PartitionSpec("core"),) * (n_params + n_outs),
                  out_specs=(PartitionSpec("core"),) * n_outs,
                  check_rep=False),
        donate_argnums=donate, keep_unused=True)
    sh = NamedSharding(mesh, PartitionSpec("core"))
    zshapes = [(NCORES * a.shape[0], *a.shape[1:]) for a in out_avals]
    zdtypes = [a.dtype for a in out_avals]
    zeros_fn = jax.jit(
        lambda: tuple(jnp.zeros(s, d) for s, d in zip(zshapes, zdtypes)),
        out_shardings=(sh,) * n_outs)

    rt = _Runtime()
    rt.in_names = in_names
    rt.out_names = out_names
    rt.sharded = sharded
    rt.zeros_fn = zeros_fn
    rt.sh = sh
    rt.devices = list(jax.devices()[:NCORES])
    _CACHE["rt"] = rt
    return rt


def _theta_host(affine_outs):
    a = affine_outs.astype(np.float64)
    sig = lambda v: 1.0 / (1.0 + np.exp(-v))
    t00 = 2 * sig(a[:, 0]); t11 = 2 * sig(a[:, 1])
    t01 = 2 * np.tanh(a[:, 2]); t10 = 2 * np.tanh(a[:, 3])
    t02 = np.tanh(a[:, 4]); t12 = np.tanh(a[:, 5])
    cx = (t00 + t01) * (0.5 - 64.0) + 64.0 * t02 + 63.5
    cy = (t10 + t11) * (0.5 - 64.0) + 64.0 * t12 + 63.5
    return t00, t01, t10, t11, cx - 32.0, cy - 32.0


def _keep_table(t00, t01, t10, t11, cxp, cyp):
    """keep[i,p,qh]: quarter-row (p, q in [32qh,32qh+32)) support test."""
    p = np.arange(128.0)
    b1 = t01[:, None] * p + cxp[:, None]
    ql1 = (-1.0 - EPS - b1) / t00[:, None]
    qh1 = (64.0 + EPS - b1) / t00[:, None]
    b2 = t11[:, None] * p + cyp[:, None]
    s = t10[:, None]
    with np.errstate(divide="ignore", invalid="ignore"):
        a2 = (-1.0 - EPS - b2) / s
        b2b = (64.0 + EPS - b2) / s
    ql2 = np.minimum(a2, b2b); qh2 = np.maximum(a2, b2b)
    tiny = np.abs(s) < 1e-12
    inr = (b2 > -1.0 - EPS) & (b2 < 64.0 + EPS)
    ql2 = np.where(tiny, np.where(inr, -1e9, 1e9), ql2)
    qh2 = np.where(tiny, np.where(inr, 1e9, -1e9), qh2)
    ql = np.maximum(ql1, ql2); qh = np.minimum(qh1, qh2)
    return np.stack([(qh >= 32.0 * h) & (ql <= 32.0 * h + 31.0)
                     for h in range(4)], axis=-1)


def _host_slots(slots, t00, t01, t10, t11, cxp, cyp, fq, sq):
    """Exact uint8-pipeline values for (ilocal, p, m, qh) overflow slots.

    fq, sq: quantized images (uint8 values as float) for the slots'
    sample set, [k?,64,64] indexed by slots[:,0].  Returns [k,32] float32
    (already /255)."""
    ii, pp, mm_, hh_ = slots.T
    qs = hh_[:, None] * 32.0 + np.arange(32.0)[None, :]      # [k,32]
    ix = t00[ii][:, None] * qs + (t01[ii] * pp + cxp[ii])[:, None]
    iy = t10[ii][:, None] * qs + (t11[ii] * pp + cyp[ii])[:, None]
    img = np.where(mm_[:, None, None] == 0, fq[ii], sq[ii])  # [k,64,64]
    x0 = np.floor(ix); y0 = np.floor(iy)
    wx = ix - x0; wy = iy - y0
    acc = np.zeros_like(ix)
    for dy in (0, 1):
        for dx in (0, 1):
            xf = x0 + dx; yf = y0 + dy
            w = (wx if dx else 1 - wx) * (wy if dy else 1 - wy)
            valid = (xf >= 0) & (xf <= 63) & (yf >= 0) & (yf <= 63)
            xi = np.clip(xf, 0, 63).astype(np.int64)
            yi = np.clip(yf, 0, 63).astype(np.int64)
            v = np.take_along_axis(
                img.reshape(img.shape[0], -1),
                (yi * 64 + xi).reshape(img.shape[0], -1), axis=1
            ).reshape(ix.shape)
            acc += np.where(valid, v, 0.0) * w
    return (np.rint(acc) * (1.0 / 63.0)).astype(np.float32)


def kernel(affine_outs, fill_alpha, stroke_alpha, targetsize):
    affine_outs = np.asarray(affine_outs, dtype=np.float32)
    fill_alpha = np.asarray(fill_alpha)
    stroke_alpha = np.asarray(stroke_alpha)
    rt = _get_runtime()
    devs = rt.devices
    half = np.float32(0.5)
    s63 = np.float32(63.0)
    t00, t01, t10, t11, cxp, cyp = _theta_host(affine_outs)
    wc6 = np.zeros((N, 8), np.float32)
    wc6[:, 0] = t01; wc6[:, 1] = t00; wc6[:, 2] = cxp
    wc6[:, 3] = t11; wc6[:, 4] = t10; wc6[:, 5] = cyp
    keep = None

    fill_out = np.zeros((N, P, P), np.float32)
    stroke_out = np.zeros((N, P, P), np.float32)
    inv = np.float32(1.0 / 63.0)
    m_of = np.empty((NS, P, 2, 4), np.int8)
    m_of[:, :, 0] = 0; m_of[:, :, 1] = 1

    # global sample index ranges: slice h, core c
    def hslice(h, c):
        return slice(c * NHALF * NS + h * NS, c * NHALF * NS + (h + 1) * NS)

    halves = []
    for h in range(NHALF):
        # quantize/pack per core-shard, dispatch each upload immediately
        ibt_shards, fqs, sqs = [], [], []
        for c in range(NCORES):
            sl = hslice(h, c)
            fq = (fill_alpha[sl] * s63 + half).astype(np.uint8)
            sq = (stroke_alpha[sl] * s63 + half).astype(np.uint8)
            fqs.append(fq); sqs.append(sq)
            codes = np.empty((NS, 64, P), np.uint8)
            codes[:, :, :64] = fq.transpose(0, 2, 1)
            codes[:, :, 64:] = sq.transpose(0, 2, 1)
            c4 = codes.reshape(NS, 64, 32, 4).astype(np.uint16)
            ibt_c = np.empty((NS, 64, 96), np.uint8)
            b3 = ibt_c.reshape(NS, 64, 32, 3)
            b3[..., 0] = ((c4[..., 0] | (c4[..., 1] << 6)) & 255).astype(np.uint8)
            b3[..., 1] = (((c4[..., 1] >> 2) | (c4[..., 2] << 4)) & 255).astype(np.uint8)
            b3[..., 2] = (((c4[..., 2] >> 4) | (c4[..., 3] << 2)) & 255).astype(np.uint8)
            ibt_shards.append(jax.device_put(ibt_c, devs[c]))
        d_ibt = jax.make_array_from_single_device_arrays(
            (NCORES * NS, 64, 96), rt.sh, ibt_shards)
        if keep is None:
            keep = _keep_table(t00, t01, t10, t11, cxp, cyp)  # [N,128,4]
        gidx = np.concatenate([np.arange(hslice(h, c).start,
                                         hslice(h, c).stop)
                               for c in range(NCORES)])
        wc6_h = wc6[gidx]
        flags_h = keep[gidx].astype(np.uint8)              # [8*ns,128,4]
        keepPG = np.repeat(keep[gidx].reshape(
            NCORES, NS, P, 1, 4), 2, axis=3)               # [8,ns,128,2,4]
        core_info = []
        for c in range(NCORES):
            kc = keepPG[c]                                 # [ns,128,2,4]
            flat = kc.reshape(-1)
            idx = np.cumsum(flat, dtype=np.int64) - 1
            over = idx >= XSLOTS
            core_info.append((kc, flat & ~over))
            ps_tot = kc.reshape(NS, -1).sum(1)
            base = np.zeros(NS, np.int64)
            base[1:] = np.cumsum(ps_tot)[:-1]
            wc6_h[c * NS:(c + 1) * NS, 6] = base.astype(np.float32)
        ins = {"ibt": d_ibt, "wc6": jax.device_put(wc6_h, rt.sh),
               "flags": jax.device_put(flags_h, rt.sh)}
        outs = rt.sharded(*[ins[name] for name in rt.in_names],
                          *rt.zeros_fn())
        arr = dict(zip(rt.out_names, outs))["comp"]
        shards = sorted(arr.addressable_shards,
                        key=lambda s: s.index[0].start or 0)
        for s in shards:
            s.data.copy_to_host_async()
        halves.append((shards, core_info, fqs, sqs))

    for h, (shards, core_info, fqs, sqs) in enumerate(halves):
        for cshard in shards:
            c = (cshard.index[0].start or 0) // XSLOTS
            kc, eff_flat = core_info[c]
            eff = eff_flat.reshape(kc.shape)
            nk = int(eff_flat.sum())
            buf = np.asarray(cshard.data)                  # [XSLOTS,24] u8
            pb = buf[:nk].reshape(nk, 8, 3).astype(np.uint16)
            cq = np.empty((nk, 8, 4), np.uint8)
            cq[..., 0] = (pb[..., 0] & 63).astype(np.uint8)
            cq[..., 1] = (((pb[..., 0] >> 6) | (pb[..., 1] << 2)) & 63).astype(np.uint8)
            cq[..., 2] = (((pb[..., 1] >> 4) | (pb[..., 2] << 4)) & 63).astype(np.uint8)
            cq[..., 3] = (pb[..., 2] >> 2).astype(np.uint8)
            vals = np.multiply(cq.reshape(nk, 32), inv, dtype=np.float32)
            sm = m_of[eff]                                 # [nk] map ids
            g0 = hslice(h, c)
            fv = fill_out[g0].reshape(NS, P, 4, 32)
            sv = stroke_out[g0].reshape(NS, P, 4, 32)
            fv[eff[:, :, 0]] = vals[sm == 0]
            sv[eff[:, :, 1]] = vals[sm == 1]
            # overflow slots (idx beyond capacity): compute on host (rare)
            dropped = kc & ~eff
            if dropped.any():
                slots = np.argwhere(dropped)
                hv = _host_slots(slots, t00[g0], t01[g0], t10[g0],
                                 t11[g0], cxp[g0], cyp[g0],
                                 fqs[c].astype(np.float64),
                                 sqs[c].astype(np.float64))
                smv = slots[:, 2]
                fv[dropped[:, :, 0]] = hv[smv == 0]
                sv[dropped[:, :, 1]] = hv[smv == 1]
    return fill_out, stroke_out

# revision 30
# speedup vs baseline: 1.0796x; 1.0796x over previous
"""Trainium2 Bass kernel for nn_AffineTransformer_6442450944616.

kernel(**inputs): FULL inputs -> (fill_out, stroke_out) [2048,128,128] f32,
matching reference.reference().  Data-parallel over samples, 8 cores.

Wall time under axon is dominated by host<->device transfer (~55-80 MB/s,
half-duplex tunnel), so the kernel minimizes transferred bytes and
overlaps everything it can:
  - images are sent as PACKED 6-bit codes (x63, 4 px per 3 bytes),
    12.6MB instead of 67MB f32; the device unpacks with strided bitwise
    DVE ops and the 255/63 rescale is folded into the output convert
    (rel err 6.1e-3 vs tolerance 2e-2, validated by simulation)
  - the output is SPARSE-COMPACTED on device: only quarter-rows (32px)
    whose bilinear-warp support is nonempty are downloaded.  The support
    is an interval per output row, computed exactly on the host from the
    affine params; the host uploads just the keep FLAGS ([ns,128,4]
    u8, map-independent) plus a per-sample slot base, and the device
    builds the slot indices itself (triangular-matmul prefix sum across
    partitions + Hillis-Steele scan over the 8 groups), then scatters
    kept quarter-rows into a compact [XSLOTS,24] uint8 tensor with
    indirect DMAs (out-of-range index = dropped slot).
    ~25% of slots are kept -> ~19MB download instead of 268MB f32 dense.
    If a core's kept slots exceed the static capacity, the overflow
    slots are computed on the host (exact, vectorized) — a rarely-taken
    safety net for input-distribution shift.
  - output values are 6-bit codes (x63), packed 4 px per 3 bytes on
    device -> download is 14.2MB; total rel err ~1.05e-2 vs tol 2e-2
  - the batch runs as two half-batch device calls (ns=128/core each) so
    device exec (dominated by ~1.2us/descriptor indirect-DMA processing)
    of one half hides under the tunnel stream of the other
  - affine params are sent as [ns,8] f32 and expanded on device; pj/qj
    pixel grids are generated on-device with iota; output donation
    buffers are created on-device (run_bass_kernel_spmd would upload
    them as host zeros every call) via a runner modeled on
    bass2jax.run_bass_via_pjrt with a cached jitted callable
  - image upload is dispatched per core-shard as soon as each shard is
    quantized; fetch+dequant+reconstruct run per shard so host work
    overlaps the tunnel stream

Math per sample i, pixel j (p=j//128, q=j%128):
  ix(j)=t00*q+t01*p+Cx ; iy likewise
  out[j] = sum_{x,y payload} relu(1-|ix-x|) * relu(1-|iy-y|) * img[y,x]
(exact bilinear-with-zeros; hat weights equal (1-w, w) on live taps).
A pixel can be nonzero only if ix in (-1,64) and iy in (-1,64); for
fixed p both are linear in q, so the support is a q-interval per row ->
the host knows exactly which quarter-rows matter (eps-margined for f32).
"""
import numpy as np
import jax
import jax.numpy as jnp
from jax.sharding import Mesh, NamedSharding, PartitionSpec
from jax.experimental.shard_map import shard_map

import concourse.bass as bass
import concourse.bacc as bacc
import concourse.tile as tile
import concourse.mybir as mybir
from concourse import bass2jax

F32 = mybir.dt.float32
BF16 = mybir.dt.bfloat16
I32 = mybir.dt.int32
U8 = mybir.dt.uint8
U16 = mybir.dt.uint16
AL = mybir.AluOpType
ACTF = mybir.ActivationFunctionType

N = 2048
NCORES = 8
NHALF = 2               # split the batch into 2 pipelined device calls
NS = N // NCORES // NHALF   # 128 samples per core per call
P = 128
NPIX = P * P
CH = 1024
NCH = NPIX // CH
XSLOTS = 19456          # compact row-pair-quarter slot capacity per core-call
EPS = 0.05              # support-interval widening (covers f32 rounding)


def _build(ns: int):
    nc = bacc.Bacc("TRN2", target_bir_lowering=False, debug=False)
    ibt_d = nc.dram_tensor("ibt", [ns, 64, 96], U8, kind="ExternalInput")
    wc6_d = nc.dram_tensor("wc6", [ns, 8], F32, kind="ExternalInput")
    flg_d = nc.dram_tensor("flags", [ns, 64, 4], U8, kind="ExternalInput")
    comp_d = nc.dram_tensor("comp", [XSLOTS, 48], U8, kind="ExternalOutput")

    with tile.TileContext(nc) as tc:
        with tc.tile_pool(name="const", bufs=1) as cpool, \
             tc.tile_pool(name="work", bufs=3) as pool, \
             tc.tile_pool(name="out", bufs=2) as opool, \
             tc.tile_pool(name="ps", bufs=2, space="PSUM") as psum, \
             tc.tile_pool(name="psw", bufs=1, space="PSUM") as psumw:
            # on-device constants: local pixel grids, chunk offsets,
            # per-partition p%64, matmul helper matrices
            pj0i = cpool.tile([P, CH], I32, tag="pj0i")
            qj0i = cpool.tile([P, CH], I32, tag="qj0i")
            c8i = cpool.tile([P, NCH], I32, tag="c8i")
            pm64i = cpool.tile([P, 1], I32, tag="pm64i")
            nc.gpsimd.iota(pj0i[:], pattern=[[2, 4], [0, 4], [1, 2], [0, 32]],
                           base=0, channel_multiplier=0)
            nc.gpsimd.iota(qj0i[:], pattern=[[0, 4], [32, 4], [0, 2], [1, 32]],
                           base=0, channel_multiplier=0)
            nc.gpsimd.iota(c8i[:], pattern=[[8, NCH]], base=0,
                           channel_multiplier=0)
            nc.gpsimd.iota(pm64i[0:64, :], pattern=[[0, 1]], base=0,
                           channel_multiplier=1)
            nc.gpsimd.iota(pm64i[64:128, :], pattern=[[0, 1]], base=0,
                           channel_multiplier=1)
            pj0 = cpool.tile([P, CH], F32, tag="pj0")
            qj0 = cpool.tile([P, CH], F32, tag="qj0")
            c8 = cpool.tile([P, NCH], F32, tag="c8")
            pm64 = cpool.tile([P, 1], F32, tag="pm64")
            nc.scalar.copy(out=pj0[:], in_=pj0i[:])
            nc.scalar.copy(out=qj0[:], in_=qj0i[:])
            nc.scalar.copy(out=c8[:], in_=c8i[:])
            nc.scalar.copy(out=pm64[:], in_=pm64i[:])
            ones2 = cpool.tile([P, 2], BF16, tag="ones2")
            nc.vector.memset(ones2[:], 0.0)
            nc.vector.memset(ones2[0:64, 0:1], 1.0)
            nc.vector.memset(ones2[64:128, 1:2], 1.0)
            one1 = cpool.tile([1, P], F32, tag="one1")
            nc.vector.memset(one1[:], 1.0)
            # triS[k,p] = (p > k): strict lower prefix matmul operand
            pfree_i = cpool.tile([P, P], I32, tag="pfree_i")
            kv_i = cpool.tile([P, 1], I32, tag="kv_i")
            nc.gpsimd.iota(pfree_i[:], pattern=[[1, P]], base=0,
                           channel_multiplier=0)
            nc.gpsimd.iota(kv_i[:], pattern=[[0, 1]], base=0,
                           channel_multiplier=1)
            pfree = cpool.tile([P, P], F32, tag="pfree")
            kv = cpool.tile([P, 1], F32, tag="kv")
            nc.scalar.copy(out=pfree[:], in_=pfree_i[:])
            nc.scalar.copy(out=kv[:], in_=kv_i[:])
            triS = cpool.tile([P, P], F32, tag="triS")
            nc.vector.tensor_scalar(triS[:], pfree[:], kv[:, 0:1], None,
                                    AL.is_gt)

            with tc.For_i(0, ns, 1) as i:
                w6 = pool.tile([1, 8], F32, tag="w6", name=f"w6{i}")
                ibtu = pool.tile([64, 96], U8, tag="ibtu", name=f"ibtu{i}")
                ftile = pool.tile([64, 4], U8, tag="ftile", name=f"ft{i}")
                nc.sync.dma_start(out=w6[:], in_=wc6_d[bass.ds(i, 1), :])
                nc.sync.dma_start(out=ibtu[:], in_=ibt_d[bass.ds(i, 1), :, :])
                nc.sync.dma_start(out=ftile[:], in_=flg_d[bass.ds(i, 1), :, :])
                # unpack 6-bit codes: 3 bytes -> 4 codes, strided views
                codes = pool.tile([64, P], U8, tag="codes", name=f"cd{i}")
                b0 = ibtu[:, 0:96:3]; b1 = ibtu[:, 1:96:3]; b2 = ibtu[:, 2:96:3]
                nc.vector.tensor_scalar(codes[:, 0:128:4], b0, 63, None,
                                        AL.bitwise_and)
                t1 = pool.tile([64, 32], U8, tag="t1", name=f"t1{i}")
                nc.vector.tensor_scalar(t1[:], b0, 6, None,
                                        AL.logical_shift_right)
                t2 = pool.tile([64, 32], U8, tag="t2", name=f"t2{i}")
                nc.vector.tensor_scalar(t2[:], b1, 15, 2, AL.bitwise_and,
                                        AL.logical_shift_left)
                nc.vector.tensor_tensor(codes[:, 1:128:4], t1[:], t2[:],
                                        AL.bitwise_or)
                t3 = pool.tile([64, 32], U8, tag="t3", name=f"t3{i}")
                nc.vector.tensor_scalar(t3[:], b1, 4, None,
                                        AL.logical_shift_right)
                t4 = pool.tile([64, 32], U8, tag="t4", name=f"t4{i}")
                nc.vector.tensor_scalar(t4[:], b2, 3, 4, AL.bitwise_and,
                                        AL.logical_shift_left)
                nc.vector.tensor_tensor(codes[:, 2:128:4], t3[:], t4[:],
                                        AL.bitwise_or)
                nc.vector.tensor_scalar(codes[:, 3:128:4], b2, 2, None,
                                        AL.logical_shift_right)
                ibtf = pool.tile([64, P], BF16, tag="ibtf", name=f"ibtf{i}")
                nc.scalar.copy(out=ibtf[:], in_=codes[:])
                # broadcast w6 row to all partitions, then select per-half
                wcb = psumw.tile([P, 8], F32, tag="wcb", name=f"wcb{i}")
                nc.tensor.matmul(out=wcb[:], lhsT=one1[:], rhs=w6[:],
                                 start=True, stop=True)
                wcs = pool.tile([P, 8], F32, tag="wcs", name=f"wcs{i}")
                nc.scalar.copy(out=wcs[:], in_=wcb[:])
                wcf0 = pool.tile([P, 1], F32, tag="wcf0", name=f"wcf0{i}")
                nc.scalar.copy(out=wcf0[0:64, :], in_=wcs[0:64, 0:1])
                nc.scalar.copy(out=wcf0[64:128, :], in_=wcs[64:128, 3:4])
                wcf1 = pool.tile([P, 1], F32, tag="wcf1", name=f"wcf1{i}")
                nc.scalar.copy(out=wcf1[0:64, :], in_=wcs[0:64, 1:2])
                nc.scalar.copy(out=wcf1[64:128, :], in_=wcs[64:128, 4:5])
                wc2f = pool.tile([P, 1], F32, tag="wc2f", name=f"wc2f{i}")
                nc.scalar.activation(out=wc2f[0:64, :], in_=pm64[0:64, :],
                                     func=ACTF.Identity, scale=-1.0,
                                     bias=wcs[0:64, 2:3])
                nc.scalar.activation(out=wc2f[64:128, :], in_=pm64[64:128, :],
                                     func=ACTF.Identity, scale=-1.0,
                                     bias=wcs[64:128, 5:6])
                # build slot indices from flags: idx(p,g) enumerates
                # (p-major, g=4m+qh) within the sample, plus the
                # per-sample base carried in wc6 col 6
                btile = pool.tile([64, 8], F32, tag="btile", name=f"bt{i}")
                nc.scalar.copy(out=btile[:, 0:4], in_=ftile[:])
                nc.scalar.copy(out=btile[:, 4:8], in_=ftile[:])
                scr8 = pool.tile([64, 8], F32, tag="scr8", name=f"sc{i}")
                rowsum = pool.tile([64, 1], F32, tag="rowsum", name=f"rs{i}")
                nc.vector.tensor_scalar(scr8[:], btile[:], 1.0, 0.0, AL.mult,
                                        AL.add, accum_out=rowsum[:])
                aux = psumw.tile([64, 1], F32, tag="aux", name=f"aux{i}")
                nc.tensor.matmul(out=aux[:], lhsT=triS[0:64, 0:64], rhs=rowsum[:],
                                 start=True, stop=True)
                rowbase = pool.tile([64, 1], F32, tag="rowbase", name=f"rb{i}")
                nc.scalar.copy(out=rowbase[:], in_=aux[:])
                s1 = pool.tile([64, 8], F32, tag="s1", name=f"s1{i}")
                nc.vector.tensor_tensor(s1[:, 1:8], btile[:, 1:8],
                                        btile[:, 0:7], AL.add)
                nc.scalar.copy(out=s1[:, 0:1], in_=btile[:, 0:1])
                s2 = pool.tile([64, 8], F32, tag="s2", name=f"s2{i}")
                nc.vector.tensor_tensor(s2[:, 2:8], s1[:, 2:8], s1[:, 0:6],
                                        AL.add)
                nc.scalar.copy(out=s2[:, 0:2], in_=s1[:, 0:2])
                s4 = pool.tile([64, 8], F32, tag="s4", name=f"s4{i}")
                nc.vector.tensor_tensor(s4[:, 4:8], s2[:, 4:8], s2[:, 0:4],
                                        AL.add)
                nc.scalar.copy(out=s4[:, 0:4], in_=s2[:, 0:4])
                excl = pool.tile([64, 8], F32, tag="excl", name=f"ex{i}")
                nc.vector.tensor_tensor(excl[:], s4[:], btile[:], AL.subtract)
                idxf = pool.tile([64, 8], F32, tag="idxf", name=f"ix{i}")
                nc.vector.tensor_scalar(idxf[:], excl[:], rowbase[:, 0:1],
                                        wcs[0:64, 6:7], AL.add, AL.add)
                msk = pool.tile([64, 8], F32, tag="msk", name=f"mk{i}")
                nc.vector.scalar_tensor_tensor(msk[:], idxf[:], float(XSLOTS),
                                               btile[:], AL.subtract, AL.mult)
                sfin = pool.tile([64, 8], F32, tag="sfin", name=f"sf{i}")
                nc.vector.tensor_scalar(sfin[:], msk[:], float(XSLOTS), None,
                                        AL.add)
                stile32 = pool.tile([64, 8], I32, tag="stile32", name=f"s32{i}")
                nc.scalar.copy(out=stile32[:], in_=sfin[:])
                # per-chunk abs-bias: wc2f + 8c*wcf0
                wc2c = pool.tile([P, NCH], F32, tag="wc2c", name=f"wc2c{i}")
                nc.scalar.activation(out=wc2c[:], in_=c8[:], func=ACTF.Identity,
                                     scale=wcf0[:], bias=wc2f[:])
                o2 = opool.tile([2, NPIX], U8, tag="o2", name=f"o2_{i}")
                for c in range(NCH):
                    d1 = pool.tile([P, CH], F32, tag="d1", name=f"d1_{c}")
                    nc.gpsimd.tensor_scalar(d1[:], pj0[:], wcf0[:], None,
                                            AL.mult)
                    d2 = pool.tile([P, CH], F32, tag="d2", name=f"d2_{c}")
                    nc.vector.scalar_tensor_tensor(d2[:], qj0[:], wcf1[:], d1[:],
                                                   AL.mult, AL.add)
                    ab = pool.tile([P, CH], F32, tag="ab", name=f"ab_{c}")
                    nc.scalar.activation(out=ab[:], in_=d2[:], func=ACTF.Abs,
                                         scale=1.0, bias=wc2c[:, c:c + 1])
                    hh = pool.tile([P, CH], BF16, tag="hh", name=f"hh_{c}")
                    nc.scalar.activation(out=hh[:], in_=ab[:], func=ACTF.Relu,
                                         scale=-1.0, bias=1.0)
                    cc = psum.tile([P, CH], F32, tag="C", name=f"cc_{c}")
                    for h in range(CH // 512):
                        hs = slice(h * 512, (h + 1) * 512)
                        nc.tensor.matmul(out=cc[:, hs], lhsT=ibtf[:], rhs=hh[0:64, hs],
                                         start=True, stop=True)
                    mm = pool.tile([P, CH], BF16, tag="mm", name=f"mm_{c}")
                    nc.vector.tensor_tensor(mm[0:64, :], cc[0:64, :], hh[64:128, :],
                                            AL.mult)
                    nc.vector.tensor_tensor(mm[64:128, :], cc[64:128, :],
                                            hh[64:128, :], AL.mult)
                    for h in range(CH // 512):
                        hs = slice(h * 512, (h + 1) * 512)
                        oo = psum.tile([2, 512], F32, tag="O", name=f"oo_{c}_{h}")
                        nc.tensor.matmul(out=oo[:], lhsT=ones2[:], rhs=mm[:, hs],
                                         start=True, stop=True)
                        # f32 PSUM -> uint8 SBUF (round-to-nearest, saturating)
                        nc.scalar.activation(out=o2[:, c * CH + h * 512:
                                                    c * CH + (h + 1) * 512],
                                             in_=oo[:], func=ACTF.Copy,
                                             scale=1.0)
                # pack 6-bit output codes: 4 px -> 3 bytes
                o2p = opool.tile([2, 12288], U8, tag="o2p", name=f"o2p_{i}")
                pc0 = o2[:, 0:NPIX:4]; pc1 = o2[:, 1:NPIX:4]
                pc2 = o2[:, 2:NPIX:4]; pc3 = o2[:, 3:NPIX:4]
                u1 = opool.tile([2, 4096], U8, tag="u1", name=f"u1_{i}")
                nc.vector.tensor_scalar(u1[:], pc1, 3, 6, AL.bitwise_and,
                                        AL.logical_shift_left)
                nc.vector.tensor_tensor(o2p[:, 0:12288:3], pc0, u1[:],
                                        AL.bitwise_or)
                u2 = opool.tile([2, 4096], U8, tag="u2", name=f"u2_{i}")
                nc.vector.tensor_scalar(u2[:], pc1, 2, None,
                                        AL.logical_shift_right)
                u3 = opool.tile([2, 4096], U8, tag="u3", name=f"u3_{i}")
                nc.vector.tensor_scalar(u3[:], pc2, 15, 4, AL.bitwise_and,
                                        AL.logical_shift_left)
                nc.vector.tensor_tensor(o2p[:, 1:12288:3], u2[:], u3[:],
                                        AL.bitwise_or)
                u4 = opool.tile([2, 4096], U8, tag="u4", name=f"u4_{i}")
                nc.vector.tensor_scalar(u4[:], pc2, 4, None,
                                        AL.logical_shift_right)
                u5 = opool.tile([2, 4096], U8, tag="u5", name=f"u5_{i}")
                nc.vector.tensor_scalar(u5[:], pc3, 2, None,
                                        AL.logical_shift_left)
                nc.vector.tensor_tensor(o2p[:, 2:12288:3], u4[:], u5[:],
                                        AL.bitwise_or)
                # reshape packed row-pairs onto partitions: with the
                # interleaved pixel order, partition rp holds
                # (qh, d, bytes) blocks; a 48B slot = quarter qh of rows
                # {2rp, 2rp+1} for one map
                o2r = opool.tile([64, 384], U8, tag="o2r", name=f"o2r_{i}")
                nc.sync.dma_start(out=o2r[:, 0:192], in_=o2p[0:1, :])
                nc.sync.dma_start(out=o2r[:, 192:384], in_=o2p[1:2, :])
                # scatter kept row-pair quarters; index XSLOTS (> bounds)
                # drops the slot.  g = 4*m + qh.
                for g in range(8):
                    nc.gpsimd.indirect_dma_start(
                        out=comp_d[:, :],
                        out_offset=bass.IndirectOffsetOnAxis(
                            ap=stile32[:, g:g + 1], axis=0),
                        in_=o2r[:, 48 * g:48 * (g + 1)],
                        in_offset=None,
                        bounds_check=XSLOTS - 1,
                        oob_is_err=False)
    nc.compile()
    return nc


class _Runtime:
    pass


_CACHE = {}


def _get_runtime() -> _Runtime:
    if "rt" in _CACHE:
        return _CACHE["rt"]
    nc = _build(NS)
    bass2jax.install_neuronx_cc_hook()
    assert nc.dbg_addr is None

    in_names, out_names, out_avals = [], [], []
    partition_name = (nc.partition_id_tensor.name
                      if nc.partition_id_tensor else None)
    for alloc in nc.m.functions[0].allocations:
        if not isinstance(alloc, mybir.MemoryLocationSet):
            continue
        name = alloc.memorylocations[0].name
        if alloc.kind == "ExternalInput":
            if name != partition_name:
                in_names.append(name)
        elif alloc.kind == "ExternalOutput":
            out_names.append(name)
            out_avals.append(jax.core.ShapedArray(
                tuple(alloc.tensor_shape), mybir.dt.np(alloc.dtype)))
    n_params = len(in_names)
    n_outs = len(out_names)
    full_in_names = list(in_names) + list(out_names)
    if partition_name is not None:
        full_in_names.append(partition_name)

    def _body(*args):
        operands = list(args)
        if partition_name is not None:
            operands.append(bass2jax.partition_id_tensor())
        outs = bass2jax._bass_exec_p.bind(
            *operands,
            out_avals=tuple(out_avals),
            in_names=tuple(full_in_names),
            out_names=tuple(out_names),
            lowering_input_output_aliases=(),
            sim_require_finite=True,
            sim_require_nnan=True,
            nc=nc,
        )
        return tuple(outs)

    mesh = Mesh(np.asarray(jax.devices()[:NCORES]), ("core",))
    donate = tuple(range(n_params, n_params + n_outs))
    sharded = jax.jit(
        shard_map(_body, mesh=mesh,
                  in_specs=(PartitionSpec("core"),) * (n_params + n_outs),
                  out_specs=(PartitionSpec("core"),) * n_outs,
                  check_rep=False),
        donate_argnums=donate, keep_unused=True)
    sh = NamedSharding(mesh, PartitionSpec("core"))
    zshapes = [(NCORES * a.shape[0], *a.shape[1:]) for a in out_avals]
    zdtypes = [a.dtype for a in out_avals]
    zeros_fn = jax.jit(
        lambda: tuple(jnp.zeros(s, d) for s, d in zip(zshapes, zdtypes)),
        out_shardings=(sh,) * n_outs)

    rt = _Runtime()
    rt.in_names = in_names
    rt.out_names = out_names
    rt.sharded = sharded
    rt.zeros_fn = zeros_fn
    rt.sh = sh
    rt.devices = list(jax.devices()[:NCORES])
    _CACHE["rt"] = rt
    return rt


def _theta_host(affine_outs):
    a = affine_outs.astype(np.float64)
    sig = lambda v: 1.0 / (1.0 + np.exp(-v))
    t00 = 2 * sig(a[:, 0]); t11 = 2 * sig(a[:, 1])
    t01 = 2 * np.tanh(a[:, 2]); t10 = 2 * np.tanh(a[:, 3])
    t02 = np.tanh(a[:, 4]); t12 = np.tanh(a[:, 5])
    cx = (t00 + t01) * (0.5 - 64.0) + 64.0 * t02 + 63.5
    cy = (t10 + t11) * (0.5 - 64.0) + 64.0 * t12 + 63.5
    return t00, t01, t10, t11, cx - 32.0, cy - 32.0


def _keep_table(t00, t01, t10, t11, cxp, cyp):
    """keep[i,p,qh]: quarter-row (p, q in [32qh,32qh+32)) support test."""
    p = np.arange(128.0)
    b1 = t01[:, None] * p + cxp[:, None]
    ql1 = (-1.0 - EPS - b1) / t00[:, None]
    qh1 = (64.0 + EPS - b1) / t00[:, None]
    b2 = t11[:, None] * p + cyp[:, None]
    s = t10[:, None]
    with np.errstate(divide="ignore", invalid="ignore"):
        a2 = (-1.0 - EPS - b2) / s
        b2b = (64.0 + EPS - b2) / s
    ql2 = np.minimum(a2, b2b); qh2 = np.maximum(a2, b2b)
    tiny = np.abs(s) < 1e-12
    inr = (b2 > -1.0 - EPS) & (b2 < 64.0 + EPS)
    ql2 = np.where(tiny, np.where(inr, -1e9, 1e9), ql2)
    qh2 = np.where(tiny, np.where(inr, 1e9, -1e9), qh2)
    ql = np.maximum(ql1, ql2); qh = np.minimum(qh1, qh2)
    return np.stack([(qh >= 32.0 * h) & (ql <= 32.0 * h + 31.0)
                     for h in range(4)], axis=-1)


def _host_slots(slots, t00, t01, t10, t11, cxp, cyp, fq, sq):
    """Exact uint8-pipeline values for (ilocal, p, m, qh) overflow slots.

    fq, sq: quantized images (uint8 values as float) for the slots'
    sample set, [k?,64,64] indexed by slots[:,0].  Returns [k,32] float32
    (already /255)."""
    ii, rp, mm_, hh_ = slots.T
    qs = hh_[:, None] * 32.0 + np.arange(32.0)[None, :]      # [k,32]
    img = np.where(mm_[:, None, None] == 0, fq[ii], sq[ii])  # [k,64,64]
    out = np.empty((len(ii), 2, 32), np.float32)
    for d in (0, 1):
        pp = 2 * rp + d
        ix = t00[ii][:, None] * qs + (t01[ii] * pp + cxp[ii])[:, None]
        iy = t10[ii][:, None] * qs + (t11[ii] * pp + cyp[ii])[:, None]
        x0 = np.floor(ix); y0 = np.floor(iy)
        wx = ix - x0; wy = iy - y0
        acc = np.zeros_like(ix)
        for dy in (0, 1):
            for dx in (0, 1):
                xf = x0 + dx; yf = y0 + dy
                w = (wx if dx else 1 - wx) * (wy if dy else 1 - wy)
                valid = (xf >= 0) & (xf <= 63) & (yf >= 0) & (yf <= 63)
                xi = np.clip(xf, 0, 63).astype(np.int64)
                yi = np.clip(yf, 0, 63).astype(np.int64)
                v = np.take_along_axis(
                    img.reshape(img.shape[0], -1),
                    (yi * 64 + xi).reshape(img.shape[0], -1), axis=1
                ).reshape(ix.shape)
                acc += np.where(valid, v, 0.0) * w
        out[:, d] = (np.rint(acc) * (1.0 / 63.0)).astype(np.float32)
    return out


def kernel(affine_outs, fill_alpha, stroke_alpha, targetsize):
    affine_outs = np.asarray(affine_outs, dtype=np.float32)
    fill_alpha = np.asarray(fill_alpha)
    stroke_alpha = np.asarray(stroke_alpha)
    rt = _get_runtime()
    devs = rt.devices
    half = np.float32(0.5)
    s63 = np.float32(63.0)
    t00, t01, t10, t11, cxp, cyp = _theta_host(affine_outs)
    wc6 = np.zeros((N, 8), np.float32)
    wc6[:, 0] = t01; wc6[:, 1] = t00; wc6[:, 2] = cxp
    wc6[:, 3] = t11; wc6[:, 4] = t10; wc6[:, 5] = cyp
    keep = None

    fill_out = np.zeros((N, P, P), np.float32)
    stroke_out = np.zeros((N, P, P), np.float32)
    inv = np.float32(1.0 / 63.0)
    m_of = np.empty((NS, 64, 2, 4), np.int8)
    m_of[:, :, 0] = 0; m_of[:, :, 1] = 1

    # global sample index ranges: slice h, core c
    def hslice(h, c):
        return slice(c * NHALF * NS + h * NS, c * NHALF * NS + (h + 1) * NS)

    halves = []
    for h in range(NHALF):
        # quantize/pack per core-shard, dispatch each upload immediately
        ibt_shards, fqs, sqs = [], [], []
        for c in range(NCORES):
            sl = hslice(h, c)
            fq = (fill_alpha[sl] * s63 + half).astype(np.uint8)
            sq = (stroke_alpha[sl] * s63 + half).astype(np.uint8)
            fqs.append(fq); sqs.append(sq)
            codes = np.empty((NS, 64, P), np.uint8)
            codes[:, :, :64] = fq.transpose(0, 2, 1)
            codes[:, :, 64:] = sq.transpose(0, 2, 1)
            c4 = codes.reshape(NS, 64, 32, 4).astype(np.uint16)
            ibt_c = np.empty((NS, 64, 96), np.uint8)
            b3 = ibt_c.reshape(NS, 64, 32, 3)
            b3[..., 0] = ((c4[..., 0] | (c4[..., 1] << 6)) & 255).astype(np.uint8)
            b3[..., 1] = (((c4[..., 1] >> 2) | (c4[..., 2] << 4)) & 255).astype(np.uint8)
            b3[..., 2] = (((c4[..., 2] >> 4) | (c4[..., 3] << 2)) & 255).astype(np.uint8)
            ibt_shards.append(jax.device_put(ibt_c, devs[c]))
        d_ibt = jax.make_array_from_single_device_arrays(
            (NCORES * NS, 64, 96), rt.sh, ibt_shards)
        if keep is None:
            keep = _keep_table(t00, t01, t10, t11, cxp, cyp)  # [N,128,4]
        gidx = np.concatenate([np.arange(hslice(h, c).start,
                                         hslice(h, c).stop)
                               for c in range(NCORES)])
        wc6_h = wc6[gidx]
        krp = keep[gidx][:, 0::2] | keep[gidx][:, 1::2]    # [8*ns,64,4]
        flags_h = krp.astype(np.uint8)
        keepPG = np.repeat(krp.reshape(
            NCORES, NS, 64, 1, 4), 2, axis=3)              # [8,ns,64,2,4]
        core_info = []
        for c in range(NCORES):
            kc = keepPG[c]                                 # [ns,128,2,4]
            flat = kc.reshape(-1)
            idx = np.cumsum(flat, dtype=np.int64) - 1
            over = idx >= XSLOTS
            core_info.append((kc, flat & ~over))
            ps_tot = kc.reshape(NS, -1).sum(1)
            base = np.zeros(NS, np.int64)
            base[1:] = np.cumsum(ps_tot)[:-1]
            wc6_h[c * NS:(c + 1) * NS, 6] = base.astype(np.float32)
        ins = {"ibt": d_ibt, "wc6": jax.device_put(wc6_h, rt.sh),
               "flags": jax.device_put(flags_h, rt.sh)}
        outs = rt.sharded(*[ins[name] for name in rt.in_names],
                          *rt.zeros_fn())
        arr = dict(zip(rt.out_names, outs))["comp"]
        shards = sorted(arr.addressable_shards,
                        key=lambda s: s.index[0].start or 0)
        for s in shards:
            s.data.copy_to_host_async()
        halves.append((shards, core_info, fqs, sqs))

    for h, (shards, core_info, fqs, sqs) in enumerate(halves):
        for cshard in shards:
            c = (cshard.index[0].start or 0) // XSLOTS
            kc, eff_flat = core_info[c]
            eff = eff_flat.reshape(kc.shape)
            nk = int(eff_flat.sum())
            buf = np.asarray(cshard.data)                  # [XSLOTS,48] u8
            pb = buf[:nk].reshape(nk, 16, 3).astype(np.uint16)
            cq = np.empty((nk, 16, 4), np.uint8)
            cq[..., 0] = (pb[..., 0] & 63).astype(np.uint8)
            cq[..., 1] = (((pb[..., 0] >> 6) | (pb[..., 1] << 2)) & 63).astype(np.uint8)
            cq[..., 2] = (((pb[..., 1] >> 4) | (pb[..., 2] << 4)) & 63).astype(np.uint8)
            cq[..., 3] = (pb[..., 2] >> 2).astype(np.uint8)
            vals = np.multiply(cq.reshape(nk, 2, 32), inv, dtype=np.float32)
            sm = m_of[eff]                                 # [nk] map ids
            g0 = hslice(h, c)
            fv = fill_out[g0].reshape(NS, 64, 2, 4, 32).transpose(0, 1, 3, 2, 4)
            sv = stroke_out[g0].reshape(NS, 64, 2, 4, 32).transpose(0, 1, 3, 2, 4)
            fv[eff[:, :, 0]] = vals[sm == 0]
            sv[eff[:, :, 1]] = vals[sm == 1]
            # overflow slots (idx beyond capacity): compute on host (rare)
            dropped = kc & ~eff
            if dropped.any():
                slots = np.argwhere(dropped)
                hv = _host_slots(slots, t00[g0], t01[g0], t10[g0],
                                 t11[g0], cxp[g0], cyp[g0],
                                 fqs[c].astype(np.float64),
                                 sqs[c].astype(np.float64))
                smv = slots[:, 2]
                fv[dropped[:, :, 0]] = hv[smv == 0]
                sv[dropped[:, :, 1]] = hv[smv == 1]
    return fill_out, stroke_out

# revision 31
# speedup vs baseline: 1.0797x; 1.0001x over previous
"""Trainium2 Bass kernel for nn_AffineTransformer_6442450944616.

kernel(**inputs): FULL inputs -> (fill_out, stroke_out) [2048,128,128] f32,
matching reference.reference().  Data-parallel over samples, 8 cores.

Wall time under axon is dominated by host<->device transfer (~55-80 MB/s,
half-duplex tunnel), so the kernel minimizes transferred bytes and
overlaps everything it can:
  - images are sent as PACKED 6-bit codes (x63, 4 px per 3 bytes),
    12.6MB instead of 67MB f32; the device unpacks with strided bitwise
    DVE ops and the 255/63 rescale is folded into the output convert
    (rel err 6.1e-3 vs tolerance 2e-2, validated by simulation)
  - the output is SPARSE-COMPACTED on device: only quarter-rows (32px)
    whose bilinear-warp support is nonempty are downloaded.  The support
    is an interval per output row, computed exactly on the host from the
    affine params; the host uploads just the keep FLAGS ([ns,128,4]
    u8, map-independent) plus a per-sample slot base, and the device
    builds the slot indices itself (triangular-matmul prefix sum across
    partitions + Hillis-Steele scan over the 8 groups), then scatters
    kept quarter-rows into a compact [XSLOTS,24] uint8 tensor with
    indirect DMAs (out-of-range index = dropped slot).
    ~25% of slots are kept -> ~19MB download instead of 268MB f32 dense.
    If a core's kept slots exceed the static capacity, the overflow
    slots are computed on the host (exact, vectorized) — a rarely-taken
    safety net for input-distribution shift.
  - output values are 6-bit codes (x63), packed 4 px per 3 bytes on
    device -> download is 14.2MB; total rel err ~1.05e-2 vs tol 2e-2
  - the batch runs as two half-batch device calls (ns=128/core each) so
    device exec (dominated by ~1.2us/descriptor indirect-DMA processing)
    of one half hides under the tunnel stream of the other
  - affine params are sent as [ns,8] f32 and expanded on device; pj/qj
    pixel grids are generated on-device with iota; output donation
    buffers are created on-device (run_bass_kernel_spmd would upload
    them as host zeros every call) via a runner modeled on
    bass2jax.run_bass_via_pjrt with a cached jitted callable
  - image upload is dispatched per core-shard as soon as each shard is
    quantized; fetch+dequant+reconstruct run per shard so host work
    overlaps the tunnel stream

Math per sample i, pixel j (p=j//128, q=j%128):
  ix(j)=t00*q+t01*p+Cx ; iy likewise
  out[j] = sum_{x,y payload} relu(1-|ix-x|) * relu(1-|iy-y|) * img[y,x]
(exact bilinear-with-zeros; hat weights equal (1-w, w) on live taps).
A pixel can be nonzero only if ix in (-1,64) and iy in (-1,64); for
fixed p both are linear in q, so the support is a q-interval per row ->
the host knows exactly which quarter-rows matter (eps-margined for f32).
"""
import numpy as np
import jax
import jax.numpy as jnp
from jax.sharding import Mesh, NamedSharding, PartitionSpec
from jax.experimental.shard_map import shard_map

import concourse.bass as bass
import concourse.bacc as bacc
import concourse.tile as tile
import concourse.mybir as mybir
from concourse import bass2jax

F32 = mybir.dt.float32
BF16 = mybir.dt.bfloat16
I32 = mybir.dt.int32
U8 = mybir.dt.uint8
U16 = mybir.dt.uint16
AL = mybir.AluOpType
ACTF = mybir.ActivationFunctionType

N = 2048
NCORES = 8
NHALF = 2               # split the batch into 2 pipelined device calls
NS = N // NCORES // NHALF   # 128 samples per core per call
P = 128
NPIX = P * P
CH = 1024
NCH = NPIX // CH
XSLOTS = 10240          # compact row-quad-quarter slot capacity per core-call
EPS = 0.05              # support-interval widening (covers f32 rounding)


def _build(ns: int):
    nc = bacc.Bacc("TRN2", target_bir_lowering=False, debug=False)
    ibt_d = nc.dram_tensor("ibt", [ns, 64, 96], U8, kind="ExternalInput")
    wc6_d = nc.dram_tensor("wc6", [ns, 8], F32, kind="ExternalInput")
    flg_d = nc.dram_tensor("flags", [ns, 32, 4], U8, kind="ExternalInput")
    comp_d = nc.dram_tensor("comp", [XSLOTS, 96], U8, kind="ExternalOutput")

    with tile.TileContext(nc) as tc:
        with tc.tile_pool(name="const", bufs=1) as cpool, \
             tc.tile_pool(name="work", bufs=3) as pool, \
             tc.tile_pool(name="out", bufs=2) as opool, \
             tc.tile_pool(name="ps", bufs=2, space="PSUM") as psum, \
             tc.tile_pool(name="psw", bufs=1, space="PSUM") as psumw:
            # on-device constants: local pixel grids, chunk offsets,
            # per-partition p%64, matmul helper matrices
            pj0i = cpool.tile([P, CH], I32, tag="pj0i")
            qj0i = cpool.tile([P, CH], I32, tag="qj0i")
            c8i = cpool.tile([P, NCH], I32, tag="c8i")
            pm64i = cpool.tile([P, 1], I32, tag="pm64i")
            nc.gpsimd.iota(pj0i[:], pattern=[[4, 2], [0, 4], [1, 4], [0, 32]],
                           base=0, channel_multiplier=0)
            nc.gpsimd.iota(qj0i[:], pattern=[[0, 2], [32, 4], [0, 4], [1, 32]],
                           base=0, channel_multiplier=0)
            nc.gpsimd.iota(c8i[:], pattern=[[8, NCH]], base=0,
                           channel_multiplier=0)
            nc.gpsimd.iota(pm64i[0:64, :], pattern=[[0, 1]], base=0,
                           channel_multiplier=1)
            nc.gpsimd.iota(pm64i[64:128, :], pattern=[[0, 1]], base=0,
                           channel_multiplier=1)
            pj0 = cpool.tile([P, CH], F32, tag="pj0")
            qj0 = cpool.tile([P, CH], F32, tag="qj0")
            c8 = cpool.tile([P, NCH], F32, tag="c8")
            pm64 = cpool.tile([P, 1], F32, tag="pm64")
            nc.scalar.copy(out=pj0[:], in_=pj0i[:])
            nc.scalar.copy(out=qj0[:], in_=qj0i[:])
            nc.scalar.copy(out=c8[:], in_=c8i[:])
            nc.scalar.copy(out=pm64[:], in_=pm64i[:])
            ones2 = cpool.tile([P, 2], BF16, tag="ones2")
            nc.vector.memset(ones2[:], 0.0)
            nc.vector.memset(ones2[0:64, 0:1], 1.0)
            nc.vector.memset(ones2[64:128, 1:2], 1.0)
            one1 = cpool.tile([1, P], F32, tag="one1")
            nc.vector.memset(one1[:], 1.0)
            # triS[k,p] = (p > k): strict lower prefix matmul operand
            pfree_i = cpool.tile([P, P], I32, tag="pfree_i")
            kv_i = cpool.tile([P, 1], I32, tag="kv_i")
            nc.gpsimd.iota(pfree_i[:], pattern=[[1, P]], base=0,
                           channel_multiplier=0)
            nc.gpsimd.iota(kv_i[:], pattern=[[0, 1]], base=0,
                           channel_multiplier=1)
            pfree = cpool.tile([P, P], F32, tag="pfree")
            kv = cpool.tile([P, 1], F32, tag="kv")
            nc.scalar.copy(out=pfree[:], in_=pfree_i[:])
            nc.scalar.copy(out=kv[:], in_=kv_i[:])
            triS = cpool.tile([P, P], F32, tag="triS")
            nc.vector.tensor_scalar(triS[:], pfree[:], kv[:, 0:1], None,
                                    AL.is_gt)

            with tc.For_i(0, ns, 1) as i:
                w6 = pool.tile([1, 8], F32, tag="w6", name=f"w6{i}")
                ibtu = pool.tile([64, 96], U8, tag="ibtu", name=f"ibtu{i}")
                ftile = pool.tile([32, 4], U8, tag="ftile", name=f"ft{i}")
                nc.sync.dma_start(out=w6[:], in_=wc6_d[bass.ds(i, 1), :])
                nc.sync.dma_start(out=ibtu[:], in_=ibt_d[bass.ds(i, 1), :, :])
                nc.sync.dma_start(out=ftile[:], in_=flg_d[bass.ds(i, 1), :, :])
                # unpack 6-bit codes: 3 bytes -> 4 codes, strided views
                codes = pool.tile([64, P], U8, tag="codes", name=f"cd{i}")
                b0 = ibtu[:, 0:96:3]; b1 = ibtu[:, 1:96:3]; b2 = ibtu[:, 2:96:3]
                nc.vector.tensor_scalar(codes[:, 0:128:4], b0, 63, None,
                                        AL.bitwise_and)
                t1 = pool.tile([64, 32], U8, tag="t1", name=f"t1{i}")
                nc.vector.tensor_scalar(t1[:], b0, 6, None,
                                        AL.logical_shift_right)
                t2 = pool.tile([64, 32], U8, tag="t2", name=f"t2{i}")
                nc.vector.tensor_scalar(t2[:], b1, 15, 2, AL.bitwise_and,
                                        AL.logical_shift_left)
                nc.vector.tensor_tensor(codes[:, 1:128:4], t1[:], t2[:],
                                        AL.bitwise_or)
                t3 = pool.tile([64, 32], U8, tag="t3", name=f"t3{i}")
                nc.vector.tensor_scalar(t3[:], b1, 4, None,
                                        AL.logical_shift_right)
                t4 = pool.tile([64, 32], U8, tag="t4", name=f"t4{i}")
                nc.vector.tensor_scalar(t4[:], b2, 3, 4, AL.bitwise_and,
                                        AL.logical_shift_left)
                nc.vector.tensor_tensor(codes[:, 2:128:4], t3[:], t4[:],
                                        AL.bitwise_or)
                nc.vector.tensor_scalar(codes[:, 3:128:4], b2, 2, None,
                                        AL.logical_shift_right)
                ibtf = pool.tile([64, P], BF16, tag="ibtf", name=f"ibtf{i}")
                nc.scalar.copy(out=ibtf[:], in_=codes[:])
                # broadcast w6 row to all partitions, then select per-half
                wcb = psumw.tile([P, 8], F32, tag="wcb", name=f"wcb{i}")
                nc.tensor.matmul(out=wcb[:], lhsT=one1[:], rhs=w6[:],
                                 start=True, stop=True)
                wcs = pool.tile([P, 8], F32, tag="wcs", name=f"wcs{i}")
                nc.scalar.copy(out=wcs[:], in_=wcb[:])
                wcf0 = pool.tile([P, 1], F32, tag="wcf0", name=f"wcf0{i}")
                nc.scalar.copy(out=wcf0[0:64, :], in_=wcs[0:64, 0:1])
                nc.scalar.copy(out=wcf0[64:128, :], in_=wcs[64:128, 3:4])
                wcf1 = pool.tile([P, 1], F32, tag="wcf1", name=f"wcf1{i}")
                nc.scalar.copy(out=wcf1[0:64, :], in_=wcs[0:64, 1:2])
                nc.scalar.copy(out=wcf1[64:128, :], in_=wcs[64:128, 4:5])
                wc2f = pool.tile([P, 1], F32, tag="wc2f", name=f"wc2f{i}")
                nc.scalar.activation(out=wc2f[0:64, :], in_=pm64[0:64, :],
                                     func=ACTF.Identity, scale=-1.0,
                                     bias=wcs[0:64, 2:3])
                nc.scalar.activation(out=wc2f[64:128, :], in_=pm64[64:128, :],
                                     func=ACTF.Identity, scale=-1.0,
                                     bias=wcs[64:128, 5:6])
                # build slot indices from flags: idx(p,g) enumerates
                # (p-major, g=4m+qh) within the sample, plus the
                # per-sample base carried in wc6 col 6
                btile = pool.tile([32, 8], F32, tag="btile", name=f"bt{i}")
                nc.scalar.copy(out=btile[:, 0:4], in_=ftile[:])
                nc.scalar.copy(out=btile[:, 4:8], in_=ftile[:])
                scr8 = pool.tile([32, 8], F32, tag="scr8", name=f"sc{i}")
                rowsum = pool.tile([32, 1], F32, tag="rowsum", name=f"rs{i}")
                nc.vector.tensor_scalar(scr8[:], btile[:], 1.0, 0.0, AL.mult,
                                        AL.add, accum_out=rowsum[:])
                aux = psumw.tile([32, 1], F32, tag="aux", name=f"aux{i}")
                nc.tensor.matmul(out=aux[:], lhsT=triS[0:32, 0:32], rhs=rowsum[:],
                                 start=True, stop=True)
                rowbase = pool.tile([32, 1], F32, tag="rowbase", name=f"rb{i}")
                nc.scalar.copy(out=rowbase[:], in_=aux[:])
                s1 = pool.tile([32, 8], F32, tag="s1", name=f"s1{i}")
                nc.vector.tensor_tensor(s1[:, 1:8], btile[:, 1:8],
                                        btile[:, 0:7], AL.add)
                nc.scalar.copy(out=s1[:, 0:1], in_=btile[:, 0:1])
                s2 = pool.tile([32, 8], F32, tag="s2", name=f"s2{i}")
                nc.vector.tensor_tensor(s2[:, 2:8], s1[:, 2:8], s1[:, 0:6],
                                        AL.add)
                nc.scalar.copy(out=s2[:, 0:2], in_=s1[:, 0:2])
                s4 = pool.tile([32, 8], F32, tag="s4", name=f"s4{i}")
                nc.vector.tensor_tensor(s4[:, 4:8], s2[:, 4:8], s2[:, 0:4],
                                        AL.add)
                nc.scalar.copy(out=s4[:, 0:4], in_=s2[:, 0:4])
                excl = pool.tile([32, 8], F32, tag="excl", name=f"ex{i}")
                nc.vector.tensor_tensor(excl[:], s4[:], btile[:], AL.subtract)
                idxf = pool.tile([32, 8], F32, tag="idxf", name=f"ix{i}")
                nc.vector.tensor_scalar(idxf[:], excl[:], rowbase[:, 0:1],
                                        wcs[0:32, 6:7], AL.add, AL.add)
                msk = pool.tile([32, 8], F32, tag="msk", name=f"mk{i}")
                nc.vector.scalar_tensor_tensor(msk[:], idxf[:], float(XSLOTS),
                                               btile[:], AL.subtract, AL.mult)
                sfin = pool.tile([32, 8], F32, tag="sfin", name=f"sf{i}")
                nc.vector.tensor_scalar(sfin[:], msk[:], float(XSLOTS), None,
                                        AL.add)
                stile32 = pool.tile([32, 8], I32, tag="stile32", name=f"s32{i}")
                nc.scalar.copy(out=stile32[:], in_=sfin[:])
                # per-chunk abs-bias: wc2f + 8c*wcf0
                wc2c = pool.tile([P, NCH], F32, tag="wc2c", name=f"wc2c{i}")
                nc.scalar.activation(out=wc2c[:], in_=c8[:], func=ACTF.Identity,
                                     scale=wcf0[:], bias=wc2f[:])
                o2 = opool.tile([2, NPIX], U8, tag="o2", name=f"o2_{i}")
                for c in range(NCH):
                    d1 = pool.tile([P, CH], F32, tag="d1", name=f"d1_{c}")
                    nc.gpsimd.tensor_scalar(d1[:], pj0[:], wcf0[:], None,
                                            AL.mult)
                    d2 = pool.tile([P, CH], F32, tag="d2", name=f"d2_{c}")
                    nc.vector.scalar_tensor_tensor(d2[:], qj0[:], wcf1[:], d1[:],
                                                   AL.mult, AL.add)
                    ab = pool.tile([P, CH], F32, tag="ab", name=f"ab_{c}")
                    nc.scalar.activation(out=ab[:], in_=d2[:], func=ACTF.Abs,
                                         scale=1.0, bias=wc2c[:, c:c + 1])
                    hh = pool.tile([P, CH], BF16, tag="hh", name=f"hh_{c}")
                    nc.scalar.activation(out=hh[:], in_=ab[:], func=ACTF.Relu,
                                         scale=-1.0, bias=1.0)
                    cc = psum.tile([P, CH], F32, tag="C", name=f"cc_{c}")
                    for h in range(CH // 512):
                        hs = slice(h * 512, (h + 1) * 512)
                        nc.tensor.matmul(out=cc[:, hs], lhsT=ibtf[:], rhs=hh[0:64, hs],
                                         start=True, stop=True)
                    mm = pool.tile([P, CH], BF16, tag="mm", name=f"mm_{c}")
                    nc.vector.tensor_tensor(mm[0:64, :], cc[0:64, :], hh[64:128, :],
                                            AL.mult)
                    nc.vector.tensor_tensor(mm[64:128, :], cc[64:128, :],
                                            hh[64:128, :], AL.mult)
                    for h in range(CH // 512):
                        hs = slice(h * 512, (h + 1) * 512)
                        oo = psum.tile([2, 512], F32, tag="O", name=f"oo_{c}_{h}")
                        nc.tensor.matmul(out=oo[:], lhsT=ones2[:], rhs=mm[:, hs],
                                         start=True, stop=True)
                        # f32 PSUM -> uint8 SBUF (round-to-nearest, saturating)
                        nc.scalar.activation(out=o2[:, c * CH + h * 512:
                                                    c * CH + (h + 1) * 512],
                                             in_=oo[:], func=ACTF.Copy,
                                             scale=1.0)
                # pack 6-bit output codes: 4 px -> 3 bytes
                o2p = opool.tile([2, 12288], U8, tag="o2p", name=f"o2p_{i}")
                pc0 = o2[:, 0:NPIX:4]; pc1 = o2[:, 1:NPIX:4]
                pc2 = o2[:, 2:NPIX:4]; pc3 = o2[:, 3:NPIX:4]
                u1 = opool.tile([2, 4096], U8, tag="u1", name=f"u1_{i}")
                nc.vector.tensor_scalar(u1[:], pc1, 3, 6, AL.bitwise_and,
                                        AL.logical_shift_left)
                nc.vector.tensor_tensor(o2p[:, 0:12288:3], pc0, u1[:],
                                        AL.bitwise_or)
                u2 = opool.tile([2, 4096], U8, tag="u2", name=f"u2_{i}")
                nc.vector.tensor_scalar(u2[:], pc1, 2, None,
                                        AL.logical_shift_right)
                u3 = opool.tile([2, 4096], U8, tag="u3", name=f"u3_{i}")
                nc.vector.tensor_scalar(u3[:], pc2, 15, 4, AL.bitwise_and,
                                        AL.logical_shift_left)
                nc.vector.tensor_tensor(o2p[:, 1:12288:3], u2[:], u3[:],
                                        AL.bitwise_or)
                u4 = opool.tile([2, 4096], U8, tag="u4", name=f"u4_{i}")
                nc.vector.tensor_scalar(u4[:], pc2, 4, None,
                                        AL.logical_shift_right)
                u5 = opool.tile([2, 4096], U8, tag="u5", name=f"u5_{i}")
                nc.vector.tensor_scalar(u5[:], pc3, 2, None,
                                        AL.logical_shift_left)
                nc.vector.tensor_tensor(o2p[:, 2:12288:3], u4[:], u5[:],
                                        AL.bitwise_or)
                # reshape packed row-pairs onto partitions: with the
                # interleaved pixel order, partition rp holds
                # (qh, d, bytes) blocks; a 48B slot = quarter qh of rows
                # {2rp, 2rp+1} for one map
                o2r = opool.tile([32, 768], U8, tag="o2r", name=f"o2r_{i}")
                nc.sync.dma_start(out=o2r[:, 0:384], in_=o2p[0:1, :])
                nc.sync.dma_start(out=o2r[:, 384:768], in_=o2p[1:2, :])
                # scatter kept row-pair quarters; index XSLOTS (> bounds)
                # drops the slot.  g = 4*m + qh.
                for g in range(8):
                    nc.gpsimd.indirect_dma_start(
                        out=comp_d[:, :],
                        out_offset=bass.IndirectOffsetOnAxis(
                            ap=stile32[:, g:g + 1], axis=0),
                        in_=o2r[:, 96 * g:96 * (g + 1)],
                        in_offset=None,
                        bounds_check=XSLOTS - 1,
                        oob_is_err=False)
    nc.compile()
    return nc


class _Runtime:
    pass


_CACHE = {}


def _get_runtime() -> _Runtime:
    if "rt" in _CACHE:
        return _CACHE["rt"]
    nc = _build(NS)
    bass2jax.install_neuronx_cc_hook()
    assert nc.dbg_addr is None

    in_names, out_names, out_avals = [], [], []
    partition_name = (nc.partition_id_tensor.name
                      if nc.partition_id_tensor else None)
    for alloc in nc.m.functions[0].allocations:
        if not isinstance(alloc, mybir.MemoryLocationSet):
            continue
        name = alloc.memorylocations[0].name
        if alloc.kind == "ExternalInput":
            if name != partition_name:
                in_names.append(name)
        elif alloc.kind == "ExternalOutput":
            out_names.append(name)
            out_avals.append(jax.core.ShapedArray(
                tuple(alloc.tensor_shape), mybir.dt.np(alloc.dtype)))
    n_params = len(in_names)
    n_outs = len(out_names)
    full_in_names = list(in_names) + list(out_names)
    if partition_name is not None:
        full_in_names.append(partition_name)

    def _body(*args):
        operands = list(args)
        if partition_name is not None:
            operands.append(bass2jax.partition_id_tensor())
        outs = bass2jax._bass_exec_p.bind(
            *operands,
            out_avals=tuple(out_avals),
            in_names=tuple(full_in_names),
            out_names=tuple(out_names),
            lowering_input_output_aliases=(),
            sim_require_finite=True,
            sim_require_nnan=True,
            nc=nc,
        )
        return tuple(outs)

    mesh = Mesh(np.asarray(jax.devices()[:NCORES]), ("core",))
    donate = tuple(range(n_params, n_params + n_outs))
    sharded = jax.jit(
        shard_map(_body, mesh=mesh,
                  in_specs=(PartitionSpec("core"),) * (n_params + n_outs),
                  out_specs=(PartitionSpec("core"),) * n_outs,
                  check_rep=False),
        donate_argnums=donate, keep_unused=True)
    sh = NamedSharding(mesh, PartitionSpec("core"))
    zshapes = [(NCORES * a.shape[0], *a.shape[1:]) for a in out_avals]
    zdtypes = [a.dtype for a in out_avals]
    zeros_fn = jax.jit(
        lambda: tuple(jnp.zeros(s, d) for s, d in zip(zshapes, zdtypes)),
        out_shardings=(sh,) * n_outs)

    rt = _Runtime()
    rt.in_names = in_names
    rt.out_names = out_names
    rt.sharded = sharded
    rt.zeros_fn = zeros_fn
    rt.sh = sh
    rt.devices = list(jax.devices()[:NCORES])
    _CACHE["rt"] = rt
    return rt


def _theta_host(affine_outs):
    a = affine_outs.astype(np.float64)
    sig = lambda v: 1.0 / (1.0 + np.exp(-v))
    t00 = 2 * sig(a[:, 0]); t11 = 2 * sig(a[:, 1])
    t01 = 2 * np.tanh(a[:, 2]); t10 = 2 * np.tanh(a[:, 3])
    t02 = np.tanh(a[:, 4]); t12 = np.tanh(a[:, 5])
    cx = (t00 + t01) * (0.5 - 64.0) + 64.0 * t02 + 63.5
    cy = (t10 + t11) * (0.5 - 64.0) + 64.0 * t12 + 63.5
    return t00, t01, t10, t11, cx - 32.0, cy - 32.0


def _keep_table(t00, t01, t10, t11, cxp, cyp):
    """keep[i,p,qh]: quarter-row (p, q in [32qh,32qh+32)) support test."""
    p = np.arange(128.0)
    b1 = t01[:, None] * p + cxp[:, None]
    ql1 = (-1.0 - EPS - b1) / t00[:, None]
    qh1 = (64.0 + EPS - b1) / t00[:, None]
    b2 = t11[:, None] * p + cyp[:, None]
    s = t10[:, None]
    with np.errstate(divide="ignore", invalid="ignore"):
        a2 = (-1.0 - EPS - b2) / s
        b2b = (64.0 + EPS - b2) / s
    ql2 = np.minimum(a2, b2b); qh2 = np.maximum(a2, b2b)
    tiny = np.abs(s) < 1e-12
    inr = (b2 > -1.0 - EPS) & (b2 < 64.0 + EPS)
    ql2 = np.where(tiny, np.where(inr, -1e9, 1e9), ql2)
    qh2 = np.where(tiny, np.where(inr, 1e9, -1e9), qh2)
    ql = np.maximum(ql1, ql2); qh = np.minimum(qh1, qh2)
    return np.stack([(qh >= 32.0 * h) & (ql <= 32.0 * h + 31.0)
                     for h in range(4)], axis=-1)


def _host_slots(slots, t00, t01, t10, t11, cxp, cyp, fq, sq):
    """Exact uint8-pipeline values for (ilocal, p, m, qh) overflow slots.

    fq, sq: quantized images (uint8 values as float) for the slots'
    sample set, [k?,64,64] indexed by slots[:,0].  Returns [k,32] float32
    (already /255)."""
    ii, rp, mm_, hh_ = slots.T
    qs = hh_[:, None] * 32.0 + np.arange(32.0)[None, :]      # [k,32]
    img = np.where(mm_[:, None, None] == 0, fq[ii], sq[ii])  # [k,64,64]
    out = np.empty((len(ii), 4, 32), np.float32)
    for d in (0, 1, 2, 3):
        pp = 4 * rp + d
        ix = t00[ii][:, None] * qs + (t01[ii] * pp + cxp[ii])[:, None]
        iy = t10[ii][:, None] * qs + (t11[ii] * pp + cyp[ii])[:, None]
        x0 = np.floor(ix); y0 = np.floor(iy)
        wx = ix - x0; wy = iy - y0
        acc = np.zeros_like(ix)
        for dy in (0, 1):
            for dx in (0, 1):
                xf = x0 + dx; yf = y0 + dy
                w = (wx if dx else 1 - wx) * (wy if dy else 1 - wy)
                valid = (xf >= 0) & (xf <= 63) & (yf >= 0) & (yf <= 63)
                xi = np.clip(xf, 0, 63).astype(np.int64)
                yi = np.clip(yf, 0, 63).astype(np.int64)
                v = np.take_along_axis(
                    img.reshape(img.shape[0], -1),
                    (yi * 64 + xi).reshape(img.shape[0], -1), axis=1
                ).reshape(ix.shape)
                acc += np.where(valid, v, 0.0) * w
        out[:, d] = (np.rint(acc) * (1.0 / 63.0)).astype(np.float32)
    return out


def kernel(affine_outs, fill_alpha, stroke_alpha, targetsize):
    affine_outs = np.asarray(affine_outs, dtype=np.float32)
    fill_alpha = np.asarray(fill_alpha)
    stroke_alpha = np.asarray(stroke_alpha)
    rt = _get_runtime()
    devs = rt.devices
    half = np.float32(0.5)
    s63 = np.float32(63.0)
    t00, t01, t10, t11, cxp, cyp = _theta_host(affine_outs)
    wc6 = np.zeros((N, 8), np.float32)
    wc6[:, 0] = t01; wc6[:, 1] = t00; wc6[:, 2] = cxp
    wc6[:, 3] = t11; wc6[:, 4] = t10; wc6[:, 5] = cyp
    keep = None

    fill_out = np.zeros((N, P, P), np.float32)
    stroke_out = np.zeros((N, P, P), np.float32)
    inv = np.float32(1.0 / 63.0)
    m_of = np.empty((NS, 32, 2, 4), np.int8)
    m_of[:, :, 0] = 0; m_of[:, :, 1] = 1

    # global sample index ranges: slice h, core c
    def hslice(h, c):
        return slice(c * NHALF * NS + h * NS, c * NHALF * NS + (h + 1) * NS)

    halves = []
    for h in range(NHALF):
        # quantize/pack per core-shard, dispatch each upload immediately
        ibt_shards, fqs, sqs = [], [], []
        for c in range(NCORES):
            sl = hslice(h, c)
            fq = (fill_alpha[sl] * s63 + half).astype(np.uint8)
            sq = (stroke_alpha[sl] * s63 + half).astype(np.uint8)
            fqs.append(fq); sqs.append(sq)
            codes = np.empty((NS, 64, P), np.uint8)
            codes[:, :, :64] = fq.transpose(0, 2, 1)
            codes[:, :, 64:] = sq.transpose(0, 2, 1)
            c4 = codes.reshape(NS, 64, 32, 4).astype(np.uint16)
            ibt_c = np.empty((NS, 64, 96), np.uint8)
            b3 = ibt_c.reshape(NS, 64, 32, 3)
            b3[..., 0] = ((c4[..., 0] | (c4[..., 1] << 6)) & 255).astype(np.uint8)
            b3[..., 1] = (((c4[..., 1] >> 2) | (c4[..., 2] << 4)) & 255).astype(np.uint8)
            b3[..., 2] = (((c4[..., 2] >> 4) | (c4[..., 3] << 2)) & 255).astype(np.uint8)
            ibt_shards.append(jax.device_put(ibt_c, devs[c]))
        d_ibt = jax.make_array_from_single_device_arrays(
            (NCORES * NS, 64, 96), rt.sh, ibt_shards)
        if keep is None:
            keep = _keep_table(t00, t01, t10, t11, cxp, cyp)  # [N,128,4]
        gidx = np.concatenate([np.arange(hslice(h, c).start,
                                         hslice(h, c).stop)
                               for c in range(NCORES)])
        wc6_h = wc6[gidx]
        kg = keep[gidx]
        krp = kg[:, 0::4] | kg[:, 1::4] | kg[:, 2::4] | kg[:, 3::4]
        flags_h = krp.astype(np.uint8)                     # [8*ns,32,4]
        keepPG = np.repeat(krp.reshape(
            NCORES, NS, 32, 1, 4), 2, axis=3)              # [8,ns,32,2,4]
        core_info = []
        for c in range(NCORES):
            kc = keepPG[c]                                 # [ns,128,2,4]
            flat = kc.reshape(-1)
            idx = np.cumsum(flat, dtype=np.int64) - 1
            over = idx >= XSLOTS
            core_info.append((kc, flat & ~over))
            ps_tot = kc.reshape(NS, -1).sum(1)
            base = np.zeros(NS, np.int64)
            base[1:] = np.cumsum(ps_tot)[:-1]
            wc6_h[c * NS:(c + 1) * NS, 6] = base.astype(np.float32)
        ins = {"ibt": d_ibt, "wc6": jax.device_put(wc6_h, rt.sh),
               "flags": jax.device_put(flags_h, rt.sh)}
        outs = rt.sharded(*[ins[name] for name in rt.in_names],
                          *rt.zeros_fn())
        arr = dict(zip(rt.out_names, outs))["comp"]
        shards = sorted(arr.addressable_shards,
                        key=lambda s: s.index[0].start or 0)
        for s in shards:
            s.data.copy_to_host_async()
        halves.append((shards, core_info, fqs, sqs))

    for h, (shards, core_info, fqs, sqs) in enumerate(halves):
        for cshard in shards:
            c = (cshard.index[0].start or 0) // XSLOTS
            kc, eff_flat = core_info[c]
            eff = eff_flat.reshape(kc.shape)
            nk = int(eff_flat.sum())
            buf = np.asarray(cshard.data)                  # [XSLOTS,96] u8
            pb = buf[:nk].reshape(nk, 32, 3).astype(np.uint16)
            cq = np.empty((nk, 32, 4), np.uint8)
            cq[..., 0] = (pb[..., 0] & 63).astype(np.uint8)
            cq[..., 1] = (((pb[..., 0] >> 6) | (pb[..., 1] << 2)) & 63).astype(np.uint8)
            cq[..., 2] = (((pb[..., 1] >> 4) | (pb[..., 2] << 4)) & 63).astype(np.uint8)
            cq[..., 3] = (pb[..., 2] >> 2).astype(np.uint8)
            vals = np.multiply(cq.reshape(nk, 4, 32), inv, dtype=np.float32)
            sm = m_of[eff]                                 # [nk] map ids
            g0 = hslice(h, c)
            fv = fill_out[g0].reshape(NS, 32, 4, 4, 32).transpose(0, 1, 3, 2, 4)
            sv = stroke_out[g0].reshape(NS, 32, 4, 4, 32).transpose(0, 1, 3, 2, 4)
            fv[eff[:, :, 0]] = vals[sm == 0]
            sv[eff[:, :, 1]] = vals[sm == 1]
            # overflow slots (idx beyond capacity): compute on host (rare)
            dropped = kc & ~eff
            if dropped.any():
                slots = np.argwhere(dropped)
                hv = _host_slots(slots, t00[g0], t01[g0], t10[g0],
                                 t11[g0], cxp[g0], cyp[g0],
                                 fqs[c].astype(np.float64),
                                 sqs[c].astype(np.float64))
                smv = slots[:, 2]
                fv[dropped[:, :, 0]] = hv[smv == 0]
                sv[dropped[:, :, 1]] = hv[smv == 1]
    return fill_out, stroke_out

# revision 32
# speedup vs baseline: 1.1992x; 1.1106x over previous
"""Trainium2 Bass kernel for nn_AffineTransformer_6442450944616.

kernel(**inputs): FULL inputs -> (fill_out, stroke_out) [2048,128,128] f32,
matching reference.reference().  Data-parallel over samples, 8 cores.

Wall time under axon is dominated by host<->device transfer (~55-80 MB/s,
half-duplex tunnel), so the kernel minimizes transferred bytes and
overlaps everything it can:
  - images are sent as PACKED 6-bit codes (x63, 4 px per 3 bytes),
    12.6MB instead of 67MB f32; the device unpacks with strided bitwise
    DVE ops and the 255/63 rescale is folded into the output convert
    (rel err 6.1e-3 vs tolerance 2e-2, validated by simulation)
  - the output is SPARSE-COMPACTED on device: only quarter-rows (32px)
    whose bilinear-warp support is nonempty are downloaded.  The support
    is an interval per output row, computed exactly on the host from the
    affine params; the host uploads just the keep FLAGS ([ns,128,4]
    u8, map-independent) plus a per-sample slot base, and the device
    builds the slot indices itself (triangular-matmul prefix sum across
    partitions + Hillis-Steele scan over the 8 groups), then scatters
    kept quarter-rows into a compact [XSLOTS,24] uint8 tensor with
    indirect DMAs (out-of-range index = dropped slot).
    ~25% of slots are kept -> ~19MB download instead of 268MB f32 dense.
    If a core's kept slots exceed the static capacity, the overflow
    slots are computed on the host (exact, vectorized) — a rarely-taken
    safety net for input-distribution shift.
  - output values are 6-bit codes (x63), packed 4 px per 3 bytes on
    device -> download is 14.2MB; total rel err ~1.05e-2 vs tol 2e-2
  - the batch runs as two half-batch device calls (ns=128/core each) so
    device exec (dominated by ~1.2us/descriptor indirect-DMA processing)
    of one half hides under the tunnel stream of the other
  - affine params are sent as [ns,8] f32 and expanded on device; pj/qj
    pixel grids are generated on-device with iota; output donation
    buffers are created on-device (run_bass_kernel_spmd would upload
    them as host zeros every call) via a runner modeled on
    bass2jax.run_bass_via_pjrt with a cached jitted callable
  - image upload is dispatched per core-shard as soon as each shard is
    quantized; fetch+dequant+reconstruct run per shard so host work
    overlaps the tunnel stream

Math per sample i, pixel j (p=j//128, q=j%128):
  ix(j)=t00*q+t01*p+Cx ; iy likewise
  out[j] = sum_{x,y payload} relu(1-|ix-x|) * relu(1-|iy-y|) * img[y,x]
(exact bilinear-with-zeros; hat weights equal (1-w, w) on live taps).
A pixel can be nonzero only if ix in (-1,64) and iy in (-1,64); for
fixed p both are linear in q, so the support is a q-interval per row ->
the host knows exactly which quarter-rows matter (eps-margined for f32).
"""
import numpy as np
import jax
import jax.numpy as jnp
from jax.sharding import Mesh, NamedSharding, PartitionSpec
from jax.experimental.shard_map import shard_map

import concourse.bass as bass
import concourse.bacc as bacc
import concourse.tile as tile
import concourse.mybir as mybir
from concourse import bass2jax

F32 = mybir.dt.float32
BF16 = mybir.dt.bfloat16
I32 = mybir.dt.int32
U8 = mybir.dt.uint8
U16 = mybir.dt.uint16
AL = mybir.AluOpType
ACTF = mybir.ActivationFunctionType

N = 2048
NCORES = 8
NHALF = 2               # split the batch into 2 pipelined device calls
NS = N // NCORES // NHALF   # 128 samples per core per call
P = 128
NPIX = P * P
CH = 1024
NCH = NPIX // CH
XSLOTS = 10240          # compact row-quad-quarter slot capacity per core-call
EPS = 0.05              # support-interval widening (covers f32 rounding)


def _build(ns: int):
    nc = bacc.Bacc("TRN2", target_bir_lowering=False, debug=False)
    ibt_d = nc.dram_tensor("ibt", [ns, 64, 96], U8, kind="ExternalInput")
    wc6_d = nc.dram_tensor("wc6", [ns, 8], F32, kind="ExternalInput")
    flg_d = nc.dram_tensor("flags", [ns, 32, 4], U8, kind="ExternalInput")
    comp_d = nc.dram_tensor("comp", [XSLOTS, 96], U8, kind="ExternalOutput")

    with tile.TileContext(nc) as tc:
        with tc.tile_pool(name="const", bufs=1) as cpool, \
             tc.tile_pool(name="work", bufs=3) as pool, \
             tc.tile_pool(name="out", bufs=2) as opool, \
             tc.tile_pool(name="ps", bufs=2, space="PSUM") as psum, \
             tc.tile_pool(name="psw", bufs=1, space="PSUM") as psumw:
            # on-device constants: local pixel grids, chunk offsets,
            # per-partition p%64, matmul helper matrices
            pj0i = cpool.tile([P, CH], I32, tag="pj0i")
            qj0i = cpool.tile([P, CH], I32, tag="qj0i")
            c8i = cpool.tile([P, NCH], I32, tag="c8i")
            pm64i = cpool.tile([P, 1], I32, tag="pm64i")
            nc.gpsimd.iota(pj0i[:], pattern=[[4, 2], [0, 4], [1, 4], [0, 32]],
                           base=0, channel_multiplier=0)
            nc.gpsimd.iota(qj0i[:], pattern=[[0, 2], [32, 4], [0, 4], [1, 32]],
                           base=0, channel_multiplier=0)
            nc.gpsimd.iota(c8i[:], pattern=[[8, NCH]], base=0,
                           channel_multiplier=0)
            nc.gpsimd.iota(pm64i[0:64, :], pattern=[[0, 1]], base=0,
                           channel_multiplier=1)
            nc.gpsimd.iota(pm64i[64:128, :], pattern=[[0, 1]], base=0,
                           channel_multiplier=1)
            pj0 = cpool.tile([P, CH], F32, tag="pj0")
            qj0 = cpool.tile([P, CH], F32, tag="qj0")
            c8 = cpool.tile([P, NCH], F32, tag="c8")
            pm64 = cpool.tile([P, 1], F32, tag="pm64")
            nc.scalar.copy(out=pj0[:], in_=pj0i[:])
            nc.scalar.copy(out=qj0[:], in_=qj0i[:])
            nc.scalar.copy(out=c8[:], in_=c8i[:])
            nc.scalar.copy(out=pm64[:], in_=pm64i[:])
            ones2 = cpool.tile([P, 2], BF16, tag="ones2")
            nc.vector.memset(ones2[:], 0.0)
            nc.vector.memset(ones2[0:64, 0:1], 1.0)
            nc.vector.memset(ones2[64:128, 1:2], 1.0)
            one1 = cpool.tile([1, P], F32, tag="one1")
            nc.vector.memset(one1[:], 1.0)
            # triS[k,p] = (p > k): strict lower prefix matmul operand
            pfree_i = cpool.tile([P, P], I32, tag="pfree_i")
            kv_i = cpool.tile([P, 1], I32, tag="kv_i")
            nc.gpsimd.iota(pfree_i[:], pattern=[[1, P]], base=0,
                           channel_multiplier=0)
            nc.gpsimd.iota(kv_i[:], pattern=[[0, 1]], base=0,
                           channel_multiplier=1)
            pfree = cpool.tile([P, P], F32, tag="pfree")
            kv = cpool.tile([P, 1], F32, tag="kv")
            nc.scalar.copy(out=pfree[:], in_=pfree_i[:])
            nc.scalar.copy(out=kv[:], in_=kv_i[:])
            triS = cpool.tile([P, P], F32, tag="triS")
            nc.vector.tensor_scalar(triS[:], pfree[:], kv[:, 0:1], None,
                                    AL.is_gt)

            with tc.For_i(0, ns, 1) as i:
                w6 = pool.tile([1, 8], F32, tag="w6", name=f"w6{i}")
                ibtu = pool.tile([64, 96], U8, tag="ibtu", name=f"ibtu{i}")
                ftile = pool.tile([32, 4], U8, tag="ftile", name=f"ft{i}")
                nc.sync.dma_start(out=w6[:], in_=wc6_d[bass.ds(i, 1), :])
                nc.sync.dma_start(out=ibtu[:], in_=ibt_d[bass.ds(i, 1), :, :])
                nc.sync.dma_start(out=ftile[:], in_=flg_d[bass.ds(i, 1), :, :])
                # unpack 6-bit codes: 3 bytes -> 4 codes, strided views
                codes = pool.tile([64, P], U8, tag="codes", name=f"cd{i}")
                b0 = ibtu[:, 0:96:3]; b1 = ibtu[:, 1:96:3]; b2 = ibtu[:, 2:96:3]
                nc.vector.tensor_scalar(codes[:, 0:128:4], b0, 63, None,
                                        AL.bitwise_and)
                t1 = pool.tile([64, 32], U8, tag="t1", name=f"t1{i}")
                nc.vector.tensor_scalar(t1[:], b0, 6, None,
                                        AL.logical_shift_right)
                t2 = pool.tile([64, 32], U8, tag="t2", name=f"t2{i}")
                nc.vector.tensor_scalar(t2[:], b1, 15, 2, AL.bitwise_and,
                                        AL.logical_shift_left)
                nc.vector.tensor_tensor(codes[:, 1:128:4], t1[:], t2[:],
                                        AL.bitwise_or)
                t3 = pool.tile([64, 32], U8, tag="t3", name=f"t3{i}")
                nc.vector.tensor_scalar(t3[:], b1, 4, None,
                                        AL.logical_shift_right)
                t4 = pool.tile([64, 32], U8, tag="t4", name=f"t4{i}")
                nc.vector.tensor_scalar(t4[:], b2, 3, 4, AL.bitwise_and,
                                        AL.logical_shift_left)
                nc.vector.tensor_tensor(codes[:, 2:128:4], t3[:], t4[:],
                                        AL.bitwise_or)
                nc.vector.tensor_scalar(codes[:, 3:128:4], b2, 2, None,
                                        AL.logical_shift_right)
                ibtf = pool.tile([64, P], BF16, tag="ibtf", name=f"ibtf{i}")
                nc.scalar.copy(out=ibtf[:], in_=codes[:])
                # broadcast w6 row to all partitions, then select per-half
                wcb = psumw.tile([P, 8], F32, tag="wcb", name=f"wcb{i}")
                nc.tensor.matmul(out=wcb[:], lhsT=one1[:], rhs=w6[:],
                                 start=True, stop=True)
                wcs = pool.tile([P, 8], F32, tag="wcs", name=f"wcs{i}")
                nc.scalar.copy(out=wcs[:], in_=wcb[:])
                wcf0 = pool.tile([P, 1], F32, tag="wcf0", name=f"wcf0{i}")
                nc.scalar.copy(out=wcf0[0:64, :], in_=wcs[0:64, 0:1])
                nc.scalar.copy(out=wcf0[64:128, :], in_=wcs[64:128, 3:4])
                wcf1 = pool.tile([P, 1], F32, tag="wcf1", name=f"wcf1{i}")
                nc.scalar.copy(out=wcf1[0:64, :], in_=wcs[0:64, 1:2])
                nc.scalar.copy(out=wcf1[64:128, :], in_=wcs[64:128, 4:5])
                wc2f = pool.tile([P, 1], F32, tag="wc2f", name=f"wc2f{i}")
                nc.scalar.activation(out=wc2f[0:64, :], in_=pm64[0:64, :],
                                     func=ACTF.Identity, scale=-1.0,
                                     bias=wcs[0:64, 2:3])
                nc.scalar.activation(out=wc2f[64:128, :], in_=pm64[64:128, :],
                                     func=ACTF.Identity, scale=-1.0,
                                     bias=wcs[64:128, 5:6])
                # build slot indices from flags: idx(p,g) enumerates
                # (p-major, g=4m+qh) within the sample, plus the
                # per-sample base carried in wc6 col 6
                btile = pool.tile([32, 8], F32, tag="btile", name=f"bt{i}")
                nc.scalar.copy(out=btile[:, 0:4], in_=ftile[:])
                nc.scalar.copy(out=btile[:, 4:8], in_=ftile[:])
                scr8 = pool.tile([32, 8], F32, tag="scr8", name=f"sc{i}")
                rowsum = pool.tile([32, 1], F32, tag="rowsum", name=f"rs{i}")
                nc.vector.tensor_scalar(scr8[:], btile[:], 1.0, 0.0, AL.mult,
                                        AL.add, accum_out=rowsum[:])
                aux = psumw.tile([32, 1], F32, tag="aux", name=f"aux{i}")
                nc.tensor.matmul(out=aux[:], lhsT=triS[0:32, 0:32], rhs=rowsum[:],
                                 start=True, stop=True)
                rowbase = pool.tile([32, 1], F32, tag="rowbase", name=f"rb{i}")
                nc.scalar.copy(out=rowbase[:], in_=aux[:])
                s1 = pool.tile([32, 8], F32, tag="s1", name=f"s1{i}")
                nc.vector.tensor_tensor(s1[:, 1:8], btile[:, 1:8],
                                        btile[:, 0:7], AL.add)
                nc.scalar.copy(out=s1[:, 0:1], in_=btile[:, 0:1])
                s2 = pool.tile([32, 8], F32, tag="s2", name=f"s2{i}")
                nc.vector.tensor_tensor(s2[:, 2:8], s1[:, 2:8], s1[:, 0:6],
                                        AL.add)
                nc.scalar.copy(out=s2[:, 0:2], in_=s1[:, 0:2])
                s4 = pool.tile([32, 8], F32, tag="s4", name=f"s4{i}")
                nc.vector.tensor_tensor(s4[:, 4:8], s2[:, 4:8], s2[:, 0:4],
                                        AL.add)
                nc.scalar.copy(out=s4[:, 0:4], in_=s2[:, 0:4])
                excl = pool.tile([32, 8], F32, tag="excl", name=f"ex{i}")
                nc.vector.tensor_tensor(excl[:], s4[:], btile[:], AL.subtract)
                idxf = pool.tile([32, 8], F32, tag="idxf", name=f"ix{i}")
                nc.vector.tensor_scalar(idxf[:], excl[:], rowbase[:, 0:1],
                                        wcs[0:32, 6:7], AL.add, AL.add)
                msk = pool.tile([32, 8], F32, tag="msk", name=f"mk{i}")
                nc.vector.scalar_tensor_tensor(msk[:], idxf[:], float(XSLOTS),
                                               btile[:], AL.subtract, AL.mult)
                sfin = pool.tile([32, 8], F32, tag="sfin", name=f"sf{i}")
                nc.vector.tensor_scalar(sfin[:], msk[:], float(XSLOTS), None,
                                        AL.add)
                stile32 = pool.tile([32, 8], I32, tag="stile32", name=f"s32{i}")
                nc.scalar.copy(out=stile32[:], in_=sfin[:])
                # per-chunk abs-bias: wc2f + 8c*wcf0
                wc2c = pool.tile([P, NCH], F32, tag="wc2c", name=f"wc2c{i}")
                nc.scalar.activation(out=wc2c[:], in_=c8[:], func=ACTF.Identity,
                                     scale=wcf0[:], bias=wc2f[:])
                o2 = opool.tile([2, NPIX], U8, tag="o2", name=f"o2_{i}")
                for c in range(NCH):
                    d1 = pool.tile([P, CH], F32, tag="d1", name=f"d1_{c}")
                    nc.gpsimd.tensor_scalar(d1[:], pj0[:], wcf0[:], None,
                                            AL.mult)
                    d2 = pool.tile([P, CH], F32, tag="d2", name=f"d2_{c}")
                    nc.vector.scalar_tensor_tensor(d2[:], qj0[:], wcf1[:], d1[:],
                                                   AL.mult, AL.add)
                    ab = pool.tile([P, CH], F32, tag="ab", name=f"ab_{c}")
                    nc.scalar.activation(out=ab[:], in_=d2[:], func=ACTF.Abs,
                                         scale=1.0, bias=wc2c[:, c:c + 1])
                    hh = pool.tile([P, CH], BF16, tag="hh", name=f"hh_{c}")
                    nc.scalar.activation(out=hh[:], in_=ab[:], func=ACTF.Relu,
                                         scale=-1.0, bias=1.0)
                    cc = psum.tile([P, CH], F32, tag="C", name=f"cc_{c}")
                    for h in range(CH // 512):
                        hs = slice(h * 512, (h + 1) * 512)
                        nc.tensor.matmul(out=cc[:, hs], lhsT=ibtf[:], rhs=hh[0:64, hs],
                                         start=True, stop=True)
                    mm = pool.tile([P, CH], BF16, tag="mm", name=f"mm_{c}")
                    nc.vector.tensor_tensor(mm[0:64, :], cc[0:64, :], hh[64:128, :],
                                            AL.mult)
                    nc.vector.tensor_tensor(mm[64:128, :], cc[64:128, :],
                                            hh[64:128, :], AL.mult)
                    for h in range(CH // 512):
                        hs = slice(h * 512, (h + 1) * 512)
                        oo = psum.tile([2, 512], F32, tag="O", name=f"oo_{c}_{h}")
                        nc.tensor.matmul(out=oo[:], lhsT=ones2[:], rhs=mm[:, hs],
                                         start=True, stop=True)
                        # f32 PSUM -> uint8 SBUF (round-to-nearest, saturating)
                        nc.scalar.activation(out=o2[:, c * CH + h * 512:
                                                    c * CH + (h + 1) * 512],
                                             in_=oo[:], func=ACTF.Copy,
                                             scale=1.0)
                # pack 6-bit output codes: 4 px -> 3 bytes
                o2p = opool.tile([2, 12288], U8, tag="o2p", name=f"o2p_{i}")
                pc0 = o2[:, 0:NPIX:4]; pc1 = o2[:, 1:NPIX:4]
                pc2 = o2[:, 2:NPIX:4]; pc3 = o2[:, 3:NPIX:4]
                u1 = opool.tile([2, 4096], U8, tag="u1", name=f"u1_{i}")
                nc.vector.tensor_scalar(u1[:], pc1, 3, 6, AL.bitwise_and,
                                        AL.logical_shift_left)
                nc.vector.tensor_tensor(o2p[:, 0:12288:3], pc0, u1[:],
                                        AL.bitwise_or)
                u2 = opool.tile([2, 4096], U8, tag="u2", name=f"u2_{i}")
                nc.vector.tensor_scalar(u2[:], pc1, 2, None,
                                        AL.logical_shift_right)
                u3 = opool.tile([2, 4096], U8, tag="u3", name=f"u3_{i}")
                nc.vector.tensor_scalar(u3[:], pc2, 15, 4, AL.bitwise_and,
                                        AL.logical_shift_left)
                nc.vector.tensor_tensor(o2p[:, 1:12288:3], u2[:], u3[:],
                                        AL.bitwise_or)
                u4 = opool.tile([2, 4096], U8, tag="u4", name=f"u4_{i}")
                nc.vector.tensor_scalar(u4[:], pc2, 4, None,
                                        AL.logical_shift_right)
                u5 = opool.tile([2, 4096], U8, tag="u5", name=f"u5_{i}")
                nc.vector.tensor_scalar(u5[:], pc3, 2, None,
                                        AL.logical_shift_left)
                nc.vector.tensor_tensor(o2p[:, 2:12288:3], u4[:], u5[:],
                                        AL.bitwise_or)
                # reshape packed row-pairs onto partitions: with the
                # interleaved pixel order, partition rp holds
                # (qh, d, bytes) blocks; a 48B slot = quarter qh of rows
                # {2rp, 2rp+1} for one map
                o2r = opool.tile([32, 768], U8, tag="o2r", name=f"o2r_{i}")
                nc.sync.dma_start(out=o2r[:, 0:384], in_=o2p[0:1, :])
                nc.sync.dma_start(out=o2r[:, 384:768], in_=o2p[1:2, :])
                # scatter kept row-pair quarters; index XSLOTS (> bounds)
                # drops the slot.  g = 4*m + qh.
                for g in range(8):
                    nc.gpsimd.indirect_dma_start(
                        out=comp_d[:, :],
                        out_offset=bass.IndirectOffsetOnAxis(
                            ap=stile32[:, g:g + 1], axis=0),
                        in_=o2r[:, 96 * g:96 * (g + 1)],
                        in_offset=None,
                        bounds_check=XSLOTS - 1,
                        oob_is_err=False)
    nc.compile()
    return nc


def _cpu_pack(fill, stroke):
    s63 = jnp.float32(63.0)
    fq = jnp.round(fill * s63).astype(jnp.uint8)
    sq = jnp.round(stroke * s63).astype(jnp.uint8)
    codes = jnp.concatenate([fq.transpose(0, 2, 1), sq.transpose(0, 2, 1)],
                            axis=2)                     # [ns,64,128]
    c4 = codes.reshape(codes.shape[0], 64, 32, 4).astype(jnp.uint16)
    b0 = (c4[..., 0] | (c4[..., 1] << 6)) & 255
    b1 = ((c4[..., 1] >> 2) | (c4[..., 2] << 4)) & 255
    b2 = ((c4[..., 2] >> 4) | (c4[..., 3] << 2)) & 255
    packed = jnp.stack([b0, b1, b2], axis=-1).astype(jnp.uint8)
    return packed.reshape(codes.shape[0], 64, 96), fq, sq


_PACK = None


def _get_pack():
    global _PACK
    if _PACK is None:
        _PACK = jax.jit(_cpu_pack, backend="cpu")
    return _PACK


class _Runtime:
    pass


_CACHE = {}


def _get_runtime() -> _Runtime:
    if "rt" in _CACHE:
        return _CACHE["rt"]
    nc = _build(NS)
    bass2jax.install_neuronx_cc_hook()
    assert nc.dbg_addr is None

    in_names, out_names, out_avals = [], [], []
    partition_name = (nc.partition_id_tensor.name
                      if nc.partition_id_tensor else None)
    for alloc in nc.m.functions[0].allocations:
        if not isinstance(alloc, mybir.MemoryLocationSet):
            continue
        name = alloc.memorylocations[0].name
        if alloc.kind == "ExternalInput":
            if name != partition_name:
                in_names.append(name)
        elif alloc.kind == "ExternalOutput":
            out_names.append(name)
            out_avals.append(jax.core.ShapedArray(
                tuple(alloc.tensor_shape), mybir.dt.np(alloc.dtype)))
    n_params = len(in_names)
    n_outs = len(out_names)
    full_in_names = list(in_names) + list(out_names)
    if partition_name is not None:
        full_in_names.append(partition_name)

    def _body(*args):
        operands = list(args)
        if partition_name is not None:
            operands.append(bass2jax.partition_id_tensor())
        outs = bass2jax._bass_exec_p.bind(
            *operands,
            out_avals=tuple(out_avals),
            in_names=tuple(full_in_names),
            out_names=tuple(out_names),
            lowering_input_output_aliases=(),
            sim_require_finite=True,
            sim_require_nnan=True,
            nc=nc,
        )
        return tuple(outs)

    mesh = Mesh(np.asarray(jax.devices()[:NCORES]), ("core",))
    donate = tuple(range(n_params, n_params + n_outs))
    sharded = jax.jit(
        shard_map(_body, mesh=mesh,
                  in_specs=(PartitionSpec("core"),) * (n_params + n_outs),
                  out_specs=(PartitionSpec("core"),) * n_outs,
                  check_rep=False),
        donate_argnums=donate, keep_unused=True)
    sh = NamedSharding(mesh, PartitionSpec("core"))
    zshapes = [(NCORES * a.shape[0], *a.shape[1:]) for a in out_avals]
    zdtypes = [a.dtype for a in out_avals]
    zeros_fn = jax.jit(
        lambda: tuple(jnp.zeros(s, d) for s, d in zip(zshapes, zdtypes)),
        out_shardings=(sh,) * n_outs)

    rt = _Runtime()
    rt.in_names = in_names
    rt.out_names = out_names
    rt.sharded = sharded
    rt.zeros_fn = zeros_fn
    rt.sh = sh
    rt.devices = list(jax.devices()[:NCORES])
    _CACHE["rt"] = rt
    return rt


def _theta_host(affine_outs):
    a = affine_outs.astype(np.float64)
    sig = lambda v: 1.0 / (1.0 + np.exp(-v))
    t00 = 2 * sig(a[:, 0]); t11 = 2 * sig(a[:, 1])
    t01 = 2 * np.tanh(a[:, 2]); t10 = 2 * np.tanh(a[:, 3])
    t02 = np.tanh(a[:, 4]); t12 = np.tanh(a[:, 5])
    cx = (t00 + t01) * (0.5 - 64.0) + 64.0 * t02 + 63.5
    cy = (t10 + t11) * (0.5 - 64.0) + 64.0 * t12 + 63.5
    return t00, t01, t10, t11, cx - 32.0, cy - 32.0


def _keep_table(t00, t01, t10, t11, cxp, cyp):
    """keep[i,p,qh]: quarter-row (p, q in [32qh,32qh+32)) support test."""
    p = np.arange(128.0)
    b1 = t01[:, None] * p + cxp[:, None]
    ql1 = (-1.0 - EPS - b1) / t00[:, None]
    qh1 = (64.0 + EPS - b1) / t00[:, None]
    b2 = t11[:, None] * p + cyp[:, None]
    s = t10[:, None]
    with np.errstate(divide="ignore", invalid="ignore"):
        a2 = (-1.0 - EPS - b2) / s
        b2b = (64.0 + EPS - b2) / s
    ql2 = np.minimum(a2, b2b); qh2 = np.maximum(a2, b2b)
    tiny = np.abs(s) < 1e-12
    inr = (b2 > -1.0 - EPS) & (b2 < 64.0 + EPS)
    ql2 = np.where(tiny, np.where(inr, -1e9, 1e9), ql2)
    qh2 = np.where(tiny, np.where(inr, 1e9, -1e9), qh2)
    ql = np.maximum(ql1, ql2); qh = np.minimum(qh1, qh2)
    return np.stack([(qh >= 32.0 * h) & (ql <= 32.0 * h + 31.0)
                     for h in range(4)], axis=-1)


def _host_slots(slots, t00, t01, t10, t11, cxp, cyp, fq, sq):
    """Exact uint8-pipeline values for (ilocal, p, m, qh) overflow slots.

    fq, sq: quantized images (uint8 values as float) for the slots'
    sample set, [k?,64,64] indexed by slots[:,0].  Returns [k,32] float32
    (already /255)."""
    ii, rp, mm_, hh_ = slots.T
    qs = hh_[:, None] * 32.0 + np.arange(32.0)[None, :]      # [k,32]
    img = np.where(mm_[:, None, None] == 0, fq[ii], sq[ii])  # [k,64,64]
    out = np.empty((len(ii), 4, 32), np.float32)
    for d in (0, 1, 2, 3):
        pp = 4 * rp + d
        ix = t00[ii][:, None] * qs + (t01[ii] * pp + cxp[ii])[:, None]
        iy = t10[ii][:, None] * qs + (t11[ii] * pp + cyp[ii])[:, None]
        x0 = np.floor(ix); y0 = np.floor(iy)
        wx = ix - x0; wy = iy - y0
        acc = np.zeros_like(ix)
        for dy in (0, 1):
            for dx in (0, 1):
                xf = x0 + dx; yf = y0 + dy
                w = (wx if dx else 1 - wx) * (wy if dy else 1 - wy)
                valid = (xf >= 0) & (xf <= 63) & (yf >= 0) & (yf <= 63)
                xi = np.clip(xf, 0, 63).astype(np.int64)
                yi = np.clip(yf, 0, 63).astype(np.int64)
                v = np.take_along_axis(
                    img.reshape(img.shape[0], -1),
                    (yi * 64 + xi).reshape(img.shape[0], -1), axis=1
                ).reshape(ix.shape)
                acc += np.where(valid, v, 0.0) * w
        out[:, d] = (np.rint(acc) * (1.0 / 63.0)).astype(np.float32)
    return out


def kernel(affine_outs, fill_alpha, stroke_alpha, targetsize):
    affine_outs = np.asarray(affine_outs, dtype=np.float32)
    fill_alpha = np.asarray(fill_alpha)
    stroke_alpha = np.asarray(stroke_alpha)
    rt = _get_runtime()
    devs = rt.devices
    half = np.float32(0.5)
    s63 = np.float32(63.0)
    t00, t01, t10, t11, cxp, cyp = _theta_host(affine_outs)
    wc6 = np.zeros((N, 8), np.float32)
    wc6[:, 0] = t01; wc6[:, 1] = t00; wc6[:, 2] = cxp
    wc6[:, 3] = t11; wc6[:, 4] = t10; wc6[:, 5] = cyp
    keep = None

    fill_out = np.zeros((N, P, P), np.float32)
    stroke_out = np.zeros((N, P, P), np.float32)
    inv = np.float32(1.0 / 63.0)
    m_of = np.empty((NS, 32, 2, 4), np.int8)
    m_of[:, :, 0] = 0; m_of[:, :, 1] = 1

    # global sample index ranges: slice h, core c
    def hslice(h, c):
        return slice(c * NHALF * NS + h * NS, c * NHALF * NS + (h + 1) * NS)

    halves = []
    for h in range(NHALF):
        # quantize/pack per core-shard, dispatch each upload immediately
        ibt_shards, fqs, sqs = [], [], []
        for c in range(NCORES):
            sl = hslice(h, c)
            ibt_c, fq, sq = _get_pack()(fill_alpha[sl], stroke_alpha[sl])
            fqs.append(np.asarray(fq)); sqs.append(np.asarray(sq))
            ibt_shards.append(jax.device_put(np.asarray(ibt_c), devs[c]))
        d_ibt = jax.make_array_from_single_device_arrays(
            (NCORES * NS, 64, 96), rt.sh, ibt_shards)
        if keep is None:
            keep = _keep_table(t00, t01, t10, t11, cxp, cyp)  # [N,128,4]
        gidx = np.concatenate([np.arange(hslice(h, c).start,
                                         hslice(h, c).stop)
                               for c in range(NCORES)])
        wc6_h = wc6[gidx]
        kg = keep[gidx]
        krp = kg[:, 0::4] | kg[:, 1::4] | kg[:, 2::4] | kg[:, 3::4]
        flags_h = krp.astype(np.uint8)                     # [8*ns,32,4]
        keepPG = np.repeat(krp.reshape(
            NCORES, NS, 32, 1, 4), 2, axis=3)              # [8,ns,32,2,4]
        core_info = []
        for c in range(NCORES):
            kc = keepPG[c]                                 # [ns,128,2,4]
            flat = kc.reshape(-1)
            idx = np.cumsum(flat, dtype=np.int64) - 1
            over = idx >= XSLOTS
            core_info.append((kc, flat & ~over))
            ps_tot = kc.reshape(NS, -1).sum(1)
            base = np.zeros(NS, np.int64)
            base[1:] = np.cumsum(ps_tot)[:-1]
            wc6_h[c * NS:(c + 1) * NS, 6] = base.astype(np.float32)
        ins = {"ibt": d_ibt, "wc6": jax.device_put(wc6_h, rt.sh),
               "flags": jax.device_put(flags_h, rt.sh)}
        outs = rt.sharded(*[ins[name] for name in rt.in_names],
                          *rt.zeros_fn())
        arr = dict(zip(rt.out_names, outs))["comp"]
        shards = sorted(arr.addressable_shards,
                        key=lambda s: s.index[0].start or 0)
        for s in shards:
            s.data.copy_to_host_async()
        halves.append((shards, core_info, fqs, sqs))

    for h, (shards, core_info, fqs, sqs) in enumerate(halves):
        for cshard in shards:
            c = (cshard.index[0].start or 0) // XSLOTS
            kc, eff_flat = core_info[c]
            eff = eff_flat.reshape(kc.shape)
            nk = int(eff_flat.sum())
            buf = np.asarray(cshard.data)                  # [XSLOTS,96] u8
            pb = buf[:nk].reshape(nk, 32, 3).astype(np.uint16)
            cq = np.empty((nk, 32, 4), np.uint8)
            cq[..., 0] = (pb[..., 0] & 63).astype(np.uint8)
            cq[..., 1] = (((pb[..., 0] >> 6) | (pb[..., 1] << 2)) & 63).astype(np.uint8)
            cq[..., 2] = (((pb[..., 1] >> 4) | (pb[..., 2] << 4)) & 63).astype(np.uint8)
            cq[..., 3] = (pb[..., 2] >> 2).astype(np.uint8)
            vals = np.multiply(cq.reshape(nk, 4, 32), inv, dtype=np.float32)
            sm = m_of[eff]                                 # [nk] map ids
            g0 = hslice(h, c)
            fv = fill_out[g0].reshape(NS, 32, 4, 4, 32).transpose(0, 1, 3, 2, 4)
            sv = stroke_out[g0].reshape(NS, 32, 4, 4, 32).transpose(0, 1, 3, 2, 4)
            fv[eff[:, :, 0]] = vals[sm == 0]
            sv[eff[:, :, 1]] = vals[sm == 1]
            # overflow slots (idx beyond capacity): compute on host (rare)
            dropped = kc & ~eff
            if dropped.any():
                slots = np.argwhere(dropped)
                hv = _host_slots(slots, t00[g0], t01[g0], t10[g0],
                                 t11[g0], cxp[g0], cyp[g0],
                                 fqs[c].astype(np.float64),
                                 sqs[c].astype(np.float64))
                smv = slots[:, 2]
                fv[dropped[:, :, 0]] = hv[smv == 0]
                sv[dropped[:, :, 1]] = hv[smv == 1]
    return fill_out, stroke_out